# revision 31
# baseline (speedup 1.0000x reference)
"""Trainium2 Bass kernel for mixed softmax + relu^2 attention (v2).

Reference computation (B=4, S=2048, D=768, H=12, DH=64):
    q = split_heads(hidden @ Wq.T + bq)        # [B,H,S,DH]
    k = split_heads(hidden @ Wk.T + bk)
    v = split_heads(hidden @ Wv.T + bv)
    scores = q @ k.T / sqrt(DH)                # [B,H,S,S]
    attn = m0 * softmax(scores) + m1 * relu(scores)^2,  (m0,m1) = softmax(w_mix)
    out = merge_heads(attn @ v) @ Wo.T + bo

Sharding over 8 NeuronCores: core = (batch b = core//2, head-group g = core%2 of
6 heads).  Each core computes its 6 heads' full SxS attention and a partial
output projection over its 384 context dims; the host sums the two partials
per batch.

Device-side layout ("transposed", k on partitions), per head pair p (2 heads
a0/a1 stacked on partitions 0-63 / 64-127):
  - qk[p] [128, 2S]: Q cols [0,S) (pre-scaled by 1/sqrt(DH) via host-side
    Wq scaling), K cols [S,2S).  Head-major rows.  Evicted from a single
    2-bank PSUM tile with one ACT copy per q-chunk.
  - scoresT tile ss [k=128, 2*512] = K_tile.T @ Q_chunk for both heads
    (row-packed concurrent matmuls via auto tile_position).
  - e = exp(ss) on ACT -> bf16; r = relu(ss)^2 on DVE (custom op) -> bf16.
  - V augmented per head: [alpha*V | beta] where alpha=max(m1,eps),
    beta=alpha/m0; e-AV accumulates [alpha*V|beta].T @ e so row 64 holds
    beta*Z (Z = softmax denominator); r-AV accumulates (alpha*V).T @ r
    col-packed for both heads into one psum tile.
  - combine: ACT evicts pse rows 0-64 -> SBUF; DVE reciprocal of the
    beta*Z rows (PSUM); GpSimd broadcasts 1/(beta*Z), multiplies and adds:
    ctx = ex * zb + xr  (equals m0*V.T e/Z + m1*V.T r by construction).
  - out_partial[o, s] = Wo_part.T @ ctx per 128-row o-tile, interleaved one
    o-tile per k-tile iteration of a later block; shipped fp32; host sums.
"""

from contextlib import ExitStack

import numpy as np
import ml_dtypes

import concourse.bass as bass
import concourse.mybir as mybir
import concourse.tile as tile
from concourse import bacc, dve_ops
from concourse.bass_utils import run_bass_kernel_spmd
from concourse.dve_spec import Spec, Src0, relu as _sp_relu, sq as _sp_sq


def _register_relu_sq():
    """Custom fused DVE op: out = relu(in0)^2 in a single pass."""
    for op in dve_ops.OPS:
        if op.name == "RELU_SQ_ANT":
            return op
    op = dve_ops.DveOp(
        "RELU_SQ_ANT",
        Spec(body=_sp_sq(_sp_relu(Src0)),
             reference=lambda in0: np.maximum(in0, 0.0) ** 2),
        subdim=False,
        uops_sha={"v3": "8abca05ebc329c1b", "v4": "4b83c053374efcdc"},
    )
    dve_ops.OPS.append(op)
    dve_ops.CUSTOM_DVE_SPECS[op.name] = op.spec
    dve_ops._SUB_OPCODE_FOR_NAME[op.name] = (
        dve_ops._CUSTOM_DVE_ROW_BASE + len(dve_ops.OPS) - 1
    )
    return op


RELU_SQ = _register_relu_sq()

B, S, D, H, DH = 4, 2048, 768, 12, 64
NCORES = 8
HL = H // 2          # local heads per core = 6
HPAIRS = HL // 2     # head pairs = 3
DLOC = HL * DH       # local context dims = 384
KTILES = S // 128    # 16
QCHUNK = 512
NQC = S // QCHUNK    # 4
DKT = D // 128       # 6 contraction tiles for projections
OTILES = D // 128    # 6 output-projection row tiles
DHP = 80             # padded per-head V block (DoubleRow needs ko stride %16==0)

F32 = mybir.dt.float32
BF16 = mybir.dt.bfloat16
F16 = mybir.dt.float16
F8 = mybir.dt.float8e4
PM = mybir.MatmulPerfMode
NP_BF16 = ml_dtypes.bfloat16
NP_F16 = np.float16
AF = mybir.ActivationFunctionType
OP = mybir.AluOpType

# AV pair j (k-tiles 2j,2j+1) issues at t-slot 2j + AVP_OFF, so the in-order
# PE stream never waits on the elementwise chain.  Pairs past the block end
# carry into the next block's first slots.
AVP_OFF = 9
# combine micro-ops start at this t-slot of the following block
CB_OFF = 4
# r-AV for k-tile t issues at t-slot t + RV_OFF
RV_OFF = 4
# engine for the outproj psum eviction: "act" | "dve" | "any"
OB_ENGINE = "act"
# engine for the psr psum eviction
XR_ENGINE = "dve"
# number of et/rt pair buffers
EW_BUFS = 6
# k-tile slots at which outproj o-tiles are emitted (on p != 0 blocks only,
# so the newest ctx q-chunk has a full block of slack before first use)
OUTPROJ_SLOTS = (12, 14, 15)

_KERNEL_CACHE: dict = {}


def build_kernel(m0: float, m1: float, has_bias: bool, repeat: int = 1):
    nc = bacc.Bacc("TRN2", target_bir_lowering=False, debug=False)

    hT = nc.dram_tensor("hT", [D, S], F16, kind="ExternalInput").ap()
    wqT = nc.dram_tensor("wqT", [D, DLOC], F16, kind="ExternalInput").ap()
    wkT = nc.dram_tensor("wkT", [D, DLOC], F16, kind="ExternalInput").ap()
    wvT = nc.dram_tensor("wvT", [D, DLOC], F16, kind="ExternalInput").ap()
    woT = nc.dram_tensor("woT", [DLOC, D], F16, kind="ExternalInput").ap()
    if has_bias:
        hb = nc.dram_tensor("hb", [1, S], F16, kind="ExternalInput").ap()
        wqb = nc.dram_tensor("wqb", [1, DLOC], F16, kind="ExternalInput").ap()
        wkb = nc.dram_tensor("wkb", [1, DLOC], F16, kind="ExternalInput").ap()
        wvb = nc.dram_tensor("wvb", [1, DLOC], F16, kind="ExternalInput").ap()
    out = nc.dram_tensor("out", [D, S], F16, kind="ExternalOutput").ap()

    # V is stored unscaled in fp8 (ones column exactly 1.0); the mix factors
    # are applied in the combine: zrec *= m0, and ctx = (xr * m1) + prod.

    with tile.TileContext(nc) as tc, ExitStack() as ctx:
        # ---------------- persistent SBUF ----------------
        pp = ctx.enter_context(tc.tile_pool(name="persist", bufs=1))

        h_t = [pp.tile([128, S], F16, tag=f"ht{k}", name=f"ht{k}") for k in range(DKT)]
        wq_t = [pp.tile([128, DLOC], F16, tag=f"wq{k}", name=f"wq{k}") for k in range(DKT)]
        wk_t = [pp.tile([128, DLOC], F16, tag=f"wk{k}", name=f"wk{k}") for k in range(DKT)]
        wv_t = [pp.tile([128, DLOC], F16, tag=f"wv{k}", name=f"wv{k}") for k in range(DKT)]
        wo_t = [pp.tile([128, D], F16, tag=f"wo{c}", name=f"wo{c}") for c in range(HPAIRS)]
        for k in range(DKT):
            nc.sync.dma_start(wk_t[k][:], wkT[k * 128:(k + 1) * 128, :])
            nc.sync.dma_start(h_t[k][:], hT[k * 128:(k + 1) * 128, :])
        for k in range(DKT):
            nc.sync.dma_start(wq_t[k][:], wqT[k * 128:(k + 1) * 128, :])
        for k in range(DKT):
            nc.sync.dma_start(wv_t[k][:], wvT[k * 128:(k + 1) * 128, :])
        for c in range(HPAIRS):
            nc.sync.dma_start(wo_t[c][:], woT[c * 128:(c + 1) * 128, :])
        if has_bias:
            hb_t = pp.tile([1, S], F16, tag="hbt")
            wqb_t = pp.tile([1, DLOC], F16, tag="wqbt")
            wkb_t = pp.tile([1, DLOC], F16, tag="wkbt")
            wvb_t = pp.tile([1, DLOC], F16, tag="wvbt")
            nc.sync.dma_start(hb_t[:], hb[:, :])
            nc.sync.dma_start(wqb_t[:], wqb[:, :])
            nc.sync.dma_start(wkb_t[:], wkb[:, :])
            nc.sync.dma_start(wvb_t[:], wvb[:, :])

        # Q and K side by side so one ACT copy evicts both per q-chunk
        qk_s = [pp.tile([128, 2 * S], F16, tag=f"qk{p}", name=f"qk{p}") for p in range(HPAIRS)]
        # V pair tiles for fp8 DoubleRow AV: vp_s[j] holds k-tiles 2j (ko=0)
        # and 2j+1 (ko=1); per head DHP cols = [V(64) | ones | pad]
        vp_s = [pp.tile([128, 2 * HL * DHP], F8, tag=f"vp{j}", name=f"vp{j}")
                for j in range(KTILES // 2)]
        # fp16 V copies for the r-branch AV (fp8 V costs ~1.5e-2 rel error)
        vb_s = [pp.tile([128, 2 * DLOC], F16, tag=f"vb{j}", name=f"vb{j}")
                for j in range(KTILES // 2)]
        # per-(p,qc) ctx tiles: avoids false tile-granular dependencies
        # between the combine write of one q-chunk and outproj reads of another
        ctx_s = [[pp.tile([128, QCHUNK], F16, tag=f"cx{p}_{q}", name=f"cx{p}_{q}")
                  for q in range(NQC)] for p in range(HPAIRS)]

        # ones columns are persistent: written once, never overwritten (the V
        # evictions write a strided AP that skips them)
        for j in range(KTILES // 2):
            vp4 = vp_s[j][:, :].rearrange("p (ko a d) -> p ko a d", ko=2, d=DHP)
            nc.gpsimd.memset(vp4[:, :, :, DH:DH + 1], 1.0)

        nkt = DKT + (1 if has_bias else 0)

        def ev_engine(name):
            if name == "act":
                return nc.scalar
            if name == "dve":
                return nc.vector
            return nc.any

        def phases(pend_outproj, outpool, obsb, rep):
            # emit one output-projection o-tile: pso accumulate over head
            # pairs, evict, dma
            def outproj_tile(qc, ot):
                cols = bass.ts(qc, QCHUNK)
                pso = outpool.tile([128, QCHUNK], F32, tag="pso",
                                   name=f"pso_r{rep}_{qc}_{ot}")
                orows = bass.ts(ot, 128)
                for c in range(HPAIRS):
                    nc.tensor.matmul(pso[:], wo_t[c][:, orows],
                                     ctx_s[c][qc][:, :],
                                     start=(c == 0), stop=(c == HPAIRS - 1))
                ob = obsb.tile([128, QCHUNK], F16, tag="ob",
                               name=f"ob_r{rep}_{qc}_{ot}")
                if ot % 2 == 0:
                    nc.scalar.activation(ob[:], pso[:], AF.Copy)
                else:
                    nc.vector.tensor_copy(ob[:], pso[:])
                nc.sync.dma_start(out[ot * 128:(ot + 1) * 128, cols], ob[:])

            def drain_outproj(n):
                while len(pend_outproj) > n:
                    qc, ot = pend_outproj.pop(0)
                    outproj_tile(qc, ot)

            # ---------------- phase 2: attention (with fused prefix) ----
            with tc.tile_pool(name="scps", bufs=2, space="PSUM") as scps, \
                 tc.tile_pool(name="acps", bufs=1, space="PSUM") as acps, \
                 tc.tile_pool(name="ewsb", bufs=EW_BUFS) as ewsb, \
                 tc.tile_pool(name="cbsb", bufs=2) as cbsb:

                def q_chain(p_, qc_):
                    # one Q projection chain [128,512] via the aux psum bank
                    ps = outpool.tile([128, QCHUNK], F32, tag="pso", name="qch")
                    for k in range(nkt):
                        rhs = h_t[k][:, bass.ts(qc_, QCHUNK)] if k < DKT \
                            else hb_t[:, bass.ts(qc_, QCHUNK)]
                        wl = wq_t[k][:, p_ * 128:(p_ + 1) * 128] if k < DKT \
                            else wqb_t[:, p_ * 128:(p_ + 1) * 128]
                        nc.tensor.matmul(ps[:], wl, rhs, start=(k == 0),
                                         stop=(k == nkt - 1))
                    nc.scalar.activation(qk_s[p_][:, qc_ * QCHUNK:(qc_ + 1) * QCHUNK],
                                         ps[:], AF.Copy)

                # ---- prefix: all K, Q(q0), all V (scores need full K; AVs
                # need V; Q(qc>0) chains are interleaved into earlier blocks)
                drain_outproj(0)
                for p_ in range(HPAIRS):
                    for g in range(2):
                        ps = scps.tile([128, 2 * QCHUNK], F32, tag="s", name="kpre")
                        for half in range(2):
                            kc = 2 * g + half
                            dst = slice(half * QCHUNK, (half + 1) * QCHUNK)
                            for k in range(nkt):
                                rhs = h_t[k][:, bass.ts(kc, QCHUNK)] if k < DKT \
                                    else hb_t[:, bass.ts(kc, QCHUNK)]
                                wl = wk_t[k][:, p_ * 128:(p_ + 1) * 128] if k < DKT \
                                    else wkb_t[:, p_ * 128:(p_ + 1) * 128]
                                nc.tensor.matmul(ps[:, dst], wl, rhs, start=(k == 0),
                                                 stop=(k == nkt - 1))
                        nc.scalar.activation(
                            qk_s[p_][:, S + 2 * g * QCHUNK:S + 2 * (g + 1) * QCHUNK],
                            ps[:], AF.Copy)
                for p_ in range(HPAIRS):
                    q_chain(p_, 0)
                for g in range(KTILES // 2):
                    ps = scps.tile([128, 2 * QCHUNK], F32, tag="s", name="vpre")
                    vp4 = vp_s[g][:, :].rearrange("p (ko a d) -> p ko a d",
                                                  ko=2, d=DHP)
                    for half in range(2):
                        t_ = 2 * g + half
                        vdst = slice(half * QCHUNK, half * QCHUNK + DLOC)
                        for k in range(nkt):
                            lhsT = h_t[k][:, bass.ts(t_, 128)] if k < DKT \
                                else hb_t[:, bass.ts(t_, 128)]
                            rhs = wv_t[k][:] if k < DKT else wvb_t[:]
                            nc.tensor.matmul(ps[:, vdst], lhsT, rhs, start=(k == 0),
                                             stop=(k == nkt - 1))
                        psv_4d = ps[:, vdst].rearrange("p (ko a d) -> p ko a d",
                                                       ko=1, d=DH)
                        nc.scalar.activation(vp4[:, half:half + 1, :, 0:DH],
                                             psv_4d[:, :, :, :], AF.Copy)
                        vb3 = vb_s[g][:, :].rearrange("p (ko x) -> p ko x", ko=2)
                        nc.scalar.activation(vb3[:, half:half + 1, :], 
                                             ps[:, vdst].rearrange("p (ko x) -> p ko x", ko=1),
                                             AF.Copy)

                def cb_step(cb, step):
                    """One micro-op of the deferred combine for the previous
                    block; spread across the next block's k-tile loop so the
                    ACT/DVE FIFOs never see a burst."""
                    if cb is None:
                        return
                    p_, pse_a, pse_b, psr_, cols_ = (
                        cb["p"], cb["pse_a"], cb["pse_b"], cb["psr"], cb["cols"])
                    if step == 0:
                        # fold the relu^2 mix weight m1 into the psr eviction
                        cb["xr"] = cbsb.tile([128, QCHUNK], F32, tag="xr", name="xr")
                        if XR_ENGINE == "act":
                            nc.scalar.activation(cb["xr"][:], psr_[:], AF.Copy, scale=m1)
                        else:
                            nc.vector.tensor_scalar_mul(cb["xr"][:], psr_[:], m1)
                    elif step == 1:
                        # fold the softmax mix weight m0 into the pse eviction
                        cb["exq"] = cbsb.tile([128, QCHUNK], F32, tag="exq", name="exq")
                        nc.scalar.activation(cb["exq"][0:64, :], pse_a[0:64, :], AF.Copy,
                                             scale=m0)
                    elif step == 2:
                        cb["zrow_a"] = cbsb.tile([1, QCHUNK], F32, tag="zwa", name="zwa")
                        nc.scalar.activation(cb["zrow_a"][0:1, :], pse_a[64:65, :], AF.Copy)
                    elif step == 3:
                        nc.scalar.activation(cb["exq"][64:128, :], pse_b[0:64, :], AF.Copy,
                                             scale=m0)
                    elif step == 4:
                        cb["zrow_b"] = cbsb.tile([1, QCHUNK], F32, tag="zwb", name="zwb")
                        nc.scalar.activation(cb["zrow_b"][0:1, :], pse_b[64:65, :], AF.Copy)
                    elif step == 5:
                        cb["zrec_a"] = cbsb.tile([1, QCHUNK], F32, tag="zra", name="zra")
                        cb["zrec_b"] = cbsb.tile([1, QCHUNK], F32, tag="zrb", name="zrb")
                        nc.vector.reciprocal_approx_fast(cb["zrec_a"][:], cb["zrow_a"][:])
                        nc.vector.reciprocal_approx_fast(cb["zrec_b"][:], cb["zrow_b"][:])
                    elif step == 7:
                        cb["zb1"] = cbsb.tile([128, QCHUNK], F32, tag="zb1", name="zb1")
                        nc.gpsimd.partition_broadcast(cb["zb1"][:, :], cb["zrec_a"][0:1, :],
                                                      channels=128)
                    elif step == 8:
                        cb["zb2"] = cbsb.tile([128, QCHUNK], F32, tag="zb2", name="zb2")
                        nc.gpsimd.partition_broadcast(cb["zb2"][:, :], cb["zrec_b"][0:1, :],
                                                      channels=128)
                    elif step == 9:
                        cb["prod"] = cbsb.tile([128, QCHUNK], F32, tag="prod", name="prod")
                        nc.gpsimd.tensor_tensor(cb["prod"][0:64, :], cb["exq"][0:64, :],
                                                cb["zb1"][0:64, :], op=OP.mult)
                    elif step == 10:
                        nc.gpsimd.tensor_tensor(cb["prod"][64:128, :], cb["exq"][64:128, :],
                                                cb["zb2"][64:128, :], op=OP.mult)
                    elif step == 11:
                        nc.gpsimd.tensor_tensor(ctx_s[p_][cb["qc"]][:, :], cb["prod"][:],
                                                cb["xr"][:], op=OP.add)
                        cb["done"] = True

                N_CB_STEPS = 12
                pending_cb = None
                pending_av = None

                for qc in range(NQC):
                    for p in range(HPAIRS):
                        a0, a1 = 2 * p, 2 * p + 1
                        cols = bass.ts(qc, QCHUNK)
                        pse_a = acps.tile([128, QCHUNK], F32, tag="peA")
                        pse_b = acps.tile([128, QCHUNK], F32, tag="peB")
                        psr = acps.tile([128, QCHUNK], F32, tag="pr")
                        pending = {}
                        pending_r = {}
                        blk = {"pse_a": pse_a, "pse_b": pse_b, "psr": psr,
                               "a0": a0, "a1": a1, "pending": pending,
                               "pending_r": pending_r}

                        def av_e_for(bk, j):
                            # fp8 DoubleRow e-AV over k-tile pair (2j, 2j+1)
                            ep = bk["pending"].pop(j)
                            st, sp = j == 0, j == KTILES // 2 - 1
                            b0, b1 = bk["a0"], bk["a1"]
                            e3 = ep[:, :].rearrange("p (ko x) -> p ko x", ko=2)
                            v3 = vp_s[j][:, :].rearrange("p (ko x) -> p ko x", ko=2)
                            va = v3[:, :, b0 * DHP:b0 * DHP + DH + 1]
                            vb = v3[:, :, b1 * DHP:b1 * DHP + DH + 1]
                            nc.tensor.matmul(bk["pse_a"][0:DH + 1, :], va, e3[:, :, 0:QCHUNK],
                                             start=st, stop=sp, perf_mode=PM.DoubleRow)
                            nc.tensor.matmul(bk["pse_b"][0:DH + 1, :], vb,
                                             e3[:, :, QCHUNK:2 * QCHUNK],
                                             start=st, stop=sp, perf_mode=PM.DoubleRow)

                        def av_r_for(bk, t):
                            # bf16 r-AV for k-tile t (col-packed head pair)
                            rt = bk["pending_r"].pop(t)
                            st, sp = t == 0, t == KTILES - 1
                            b0, b1 = bk["a0"], bk["a1"]
                            j, ph = t // 2, t % 2
                            v3 = vb_s[j][:, :].rearrange("p (ko x) -> p ko x", ko=2)
                            nc.tensor.matmul(bk["psr"][0:64, :],
                                             v3[:, ph:ph + 1, b0 * DH:(b0 + 1) * DH],
                                             rt[:, 0:QCHUNK], start=st, stop=sp)
                            nc.tensor.matmul(bk["psr"][64:128, :],
                                             v3[:, ph:ph + 1, b1 * DH:(b1 + 1) * DH],
                                             rt[:, QCHUNK:2 * QCHUNK], start=st, stop=sp)

                        for t in range(KTILES):
                            kcols = slice(S + t * 128, S + (t + 1) * 128)
                            qcols = slice(qc * QCHUNK, (qc + 1) * QCHUNK)
                            # both heads' score tiles side by side in one 2-bank
                            # PSUM tile; the two matmuls row-pack (tile_position
                            # (0,0) and (64,0) via base partitions)
                            ss = scps.tile([128, 2 * QCHUNK], F32, tag="s")
                            nc.tensor.matmul(ss[:, 0:QCHUNK], qk_s[p][0:64, kcols],
                                             qk_s[p][0:64, qcols])
                            nc.tensor.matmul(ss[:, QCHUNK:2 * QCHUNK], qk_s[p][64:128, kcols],
                                             qk_s[p][64:128, qcols])

                            # e pair tiles fp8 (ko-major halves); r tiles bf16
                            if t % 2 == 0:
                                etp = ewsb.tile([128, 4 * QCHUNK], F8, tag="e", name="e")
                                pending[t // 2] = etp
                            half = slice((t % 2) * 2 * QCHUNK, (t % 2 + 1) * 2 * QCHUNK)
                            rt = ewsb.tile([128, 2 * QCHUNK], F16, tag="r", name="r")
                            pending_r[t] = rt
                            nc.scalar.activation(etp[:, half], ss[:], AF.Exp)
                            nc.vector._custom_dve(RELU_SQ, out=rt[:], in0=ss[:])
                            # previous block's tail AVs, spread over early slots
                            if pending_av is not None:
                                bk_p, etail, rtail = pending_av
                                if t < len(etail):
                                    av_e_for(bk_p, etail[t])
                                if t < len(rtail):
                                    av_r_for(bk_p, rtail[t])
                            # previous block's combine, one micro-op per tile
                            cb_step(pending_cb, t - CB_OFF)
                            if t >= AVP_OFF and (t - AVP_OFF) % 2 == 0:
                                av_e_for(blk, (t - AVP_OFF) // 2)
                            if t >= RV_OFF:
                                av_r_for(blk, t - RV_OFF)
                            # next q-chunk's Q projection, one chain per block
                            if t == 5 and qc + 1 < NQC:
                                q_chain(p, qc + 1)
                            # pending outproj o-tiles mid-block
                            if p != 0 and t in OUTPROJ_SLOTS and pend_outproj:
                                qc_o, ot_o = pend_outproj.pop(0)
                                outproj_tile(qc_o, ot_o)
                        pending_av_next = (blk, sorted(pending), sorted(pending_r))

                        # defer this block's tail AVs and combine into the
                        # next block's loop
                        pending_av = pending_av_next
                        pending_cb = {"p": p, "pse_a": pse_a, "pse_b": pse_b,
                                      "psr": psr, "cols": cols, "qc": qc}

                        # queue this q-chunk's output projection once all head
                        # pairs are done; emitted interleaved in later blocks
                        if p == HPAIRS - 1:
                            for ot in range(OTILES):
                                pend_outproj.append((qc, ot))

                # drain the last block's tail AVs and combine
                if pending_av is not None:
                    for j in pending_av[1]:
                        av_e_for(pending_av[0], j)
                    for tt in pending_av[2]:
                        av_r_for(pending_av[0], tt)
                for st_i in range(N_CB_STEPS):
                    cb_step(pending_cb, st_i)

        pend_outproj: list = []
        with tc.tile_pool(name="outps", bufs=1, space="PSUM") as outpool, \
             tc.tile_pool(name="obsb", bufs=2) as obsb:
            for _rep in range(repeat):
                phases(pend_outproj, outpool, obsb, _rep)
            # tail: remaining outproj tiles of the last rep (phase pools are
            # closed here, so banks are free for a wider tail pool)
            with tc.tile_pool(name="tailps", bufs=3, space="PSUM") as tailpool:
                while pend_outproj:
                    qc, ot = pend_outproj.pop(0)
                    cols = bass.ts(qc, QCHUNK)
                    pso = tailpool.tile([128, QCHUNK], F32, tag="pso",
                                        name=f"pso_tail_{qc}_{ot}")
                    orows = bass.ts(ot, 128)
                    for c in range(HPAIRS):
                        nc.tensor.matmul(pso[:], wo_t[c][:, orows],
                                         ctx_s[c][qc][:, :],
                                         start=(c == 0), stop=(c == HPAIRS - 1))
                    ob = obsb.tile([128, QCHUNK], F16, tag="ob",
                                   name=f"ob_tail_{qc}_{ot}")
                    if ot % 2 == 0:
                        nc.scalar.activation(ob[:], pso[:], AF.Copy)
                    else:
                        nc.vector.tensor_copy(ob[:], pso[:])
                    nc.sync.dma_start(out[ot * 128:(ot + 1) * 128, cols], ob[:])

    nc.compile()
    return nc


def _get_kernel(m0: float, m1: float, has_bias: bool):
    key = (round(m0, 9), round(m1, 9), has_bias)
    if key not in _KERNEL_CACHE:
        _KERNEL_CACHE[key] = build_kernel(m0, m1, has_bias)
    return _KERNEL_CACHE[key]


def make_in_maps(inputs: dict) -> tuple[list[dict], float, float, bool]:
    hidden = np.asarray(inputs["hidden_states"], dtype=np.float32)
    Wq = np.asarray(inputs["Wq"], dtype=np.float32)
    Wk = np.asarray(inputs["Wk"], dtype=np.float32)
    Wv = np.asarray(inputs["Wv"], dtype=np.float32)
    Wo = np.asarray(inputs["Wo"], dtype=np.float32)
    bq = np.asarray(inputs["bq"], dtype=np.float32)
    bk = np.asarray(inputs["bk"], dtype=np.float32)
    bv = np.asarray(inputs["bv"], dtype=np.float32)
    w_mix = np.asarray(inputs["w_mix"], dtype=np.float32)

    e = np.exp(w_mix - w_mix.max())
    mix = e / e.sum()
    m0, m1 = float(mix[0]), float(mix[1])
    has_bias = bool(bq.any() or bk.any() or bv.any())

    qk_scale = 1.0 / float(np.sqrt(DH))

    def bf(x):
        return np.ascontiguousarray(x).astype(NP_F16)

    in_maps = []
    for core in range(NCORES):
        b, g = core // 2, core % 2
        rows = slice(DLOC * g, DLOC * (g + 1))
        m = {
            "hT": bf(hidden[b].T),
            "wqT": bf(Wq[rows].T * qk_scale),
            "wkT": bf(Wk[rows].T),
            "wvT": bf(Wv[rows].T),
            "woT": bf(Wo[:, rows].T),
        }
        if has_bias:
            m["hb"] = bf(np.ones((1, S), dtype=np.float32))
            m["wqb"] = bf(bq[rows][None, :] * qk_scale)
            m["wkb"] = bf(bk[rows][None, :])
            m["wvb"] = bf(bv[rows][None, :])
        in_maps.append(m)
    return in_maps, m0, m1, has_bias


def assemble_output(results: list[dict], bo: np.ndarray) -> np.ndarray:
    out = np.empty((B, S, D), dtype=np.float32)
    for b in range(B):
        out[b] = (results[2 * b]["out"].astype(np.float32) +
                  results[2 * b + 1]["out"].astype(np.float32)).T
    if bo.any():
        out += bo
    return out


def _spot_check(out: np.ndarray, inputs: dict, rng: np.random.Generator) -> bool:
    """Recompute one random query row per batch on the host (covers all 8
    cores' partial outputs) and compare; guards against transient HW faults."""
    hidden = np.asarray(inputs["hidden_states"], dtype=np.float32)
    Wq = np.asarray(inputs["Wq"], dtype=np.float32)
    Wk = np.asarray(inputs["Wk"], dtype=np.float32)
    Wv = np.asarray(inputs["Wv"], dtype=np.float32)
    Wo = np.asarray(inputs["Wo"], dtype=np.float32)
    bq = np.asarray(inputs["bq"], dtype=np.float32)
    bk = np.asarray(inputs["bk"], dtype=np.float32)
    bv = np.asarray(inputs["bv"], dtype=np.float32)
    bo = np.asarray(inputs["bo"], dtype=np.float32)
    w_mix = np.asarray(inputs["w_mix"], dtype=np.float32)
    e = np.exp(w_mix - w_mix.max())
    m0, m1 = e / e.sum()
    for b in range(B):
        s = int(rng.integers(0, S))
        q = (hidden[b, s] @ Wq.T + bq).reshape(H, DH) / np.sqrt(DH)
        k = (hidden[b] @ Wk.T + bk).reshape(S, H, DH)
        v = (hidden[b] @ Wv.T + bv).reshape(S, H, DH)
        scores = np.einsum("hd,khd->hk", q, k)
        sm = np.exp(scores - scores.max(axis=1, keepdims=True))
        sm /= sm.sum(axis=1, keepdims=True)
        attn = m0 * sm + m1 * np.maximum(scores, 0.0) ** 2
        ctx = np.einsum("hk,khd->hd", attn, v).reshape(D)
        want = ctx @ Wo.T + bo
        got = out[b, s]
        rel = np.abs(got - want).max() / max(np.abs(want).max(), 1e-6)
        if not np.isfinite(got).all() or rel > 0.05:
            return False
    return True


def kernel(**inputs) -> np.ndarray:
    in_maps, m0, m1, has_bias = make_in_maps(inputs)
    nc = _get_kernel(m0, m1, has_bias)
    bo = np.asarray(inputs["bo"], dtype=np.float32)
    rng = np.random.default_rng(12345)
    out = None
    for _attempt in range(3):
        res = run_bass_kernel_spmd(nc, in_maps, core_ids=list(range(NCORES)))
        out = assemble_output(res.results, bo)
        if np.isfinite(out).all() and _spot_check(out, inputs, rng):
            return out
    return out


# revision 33
# speedup vs baseline: 1.5385x; 1.5385x over previous
"""Trainium2 Bass kernel for mixed softmax + relu^2 attention (v2).

Reference computation (B=4, S=2048, D=768, H=12, DH=64):
    q = split_heads(hidden @ Wq.T + bq)        # [B,H,S,DH]
    k = split_heads(hidden @ Wk.T + bk)
    v = split_heads(hidden @ Wv.T + bv)
    scores = q @ k.T / sqrt(DH)                # [B,H,S,S]
    attn = m0 * softmax(scores) + m1 * relu(scores)^2,  (m0,m1) = softmax(w_mix)
    out = merge_heads(attn @ v) @ Wo.T + bo

Sharding over 8 NeuronCores: core = (batch b = core//2, head-group g = core%2 of
6 heads).  Each core computes its 6 heads' full SxS attention and a partial
output projection over its 384 context dims; the host sums the two partials
per batch.

Device-side layout ("transposed", k on partitions), per head pair p (2 heads
a0/a1 stacked on partitions 0-63 / 64-127):
  - qk[p] [128, 2S]: Q cols [0,S) (pre-scaled by 1/sqrt(DH) via host-side
    Wq scaling), K cols [S,2S).  Head-major rows.  Evicted from a single
    2-bank PSUM tile with one ACT copy per q-chunk.
  - scoresT tile ss [k=128, 2*512] = K_tile.T @ Q_chunk for both heads
    (row-packed concurrent matmuls via auto tile_position).
  - e = exp(ss) on ACT -> bf16; r = relu(ss)^2 on DVE (custom op) -> bf16.
  - V augmented per head: [alpha*V | beta] where alpha=max(m1,eps),
    beta=alpha/m0; e-AV accumulates [alpha*V|beta].T @ e so row 64 holds
    beta*Z (Z = softmax denominator); r-AV accumulates (alpha*V).T @ r
    col-packed for both heads into one psum tile.
  - combine: ACT evicts pse rows 0-64 -> SBUF; DVE reciprocal of the
    beta*Z rows (PSUM); GpSimd broadcasts 1/(beta*Z), multiplies and adds:
    ctx = ex * zb + xr  (equals m0*V.T e/Z + m1*V.T r by construction).
  - out_partial[o, s] = Wo_part.T @ ctx per 128-row o-tile, interleaved one
    o-tile per k-tile iteration of a later block; shipped fp32; host sums.
"""

from contextlib import ExitStack

import numpy as np
import ml_dtypes

import concourse.bass as bass
import concourse.mybir as mybir
import concourse.tile as tile
from concourse import bacc, dve_ops
from concourse.bass_utils import run_bass_kernel_spmd
from concourse.dve_spec import Spec, Src0, relu as _sp_relu, sq as _sp_sq


def _register_relu_sq():
    """Custom fused DVE op: out = relu(in0)^2 in a single pass."""
    for op in dve_ops.OPS:
        if op.name == "RELU_SQ_ANT":
            return op
    op = dve_ops.DveOp(
        "RELU_SQ_ANT",
        Spec(body=_sp_sq(_sp_relu(Src0)),
             reference=lambda in0: np.maximum(in0, 0.0) ** 2),
        subdim=False,
        uops_sha={"v3": "8abca05ebc329c1b", "v4": "4b83c053374efcdc"},
    )
    dve_ops.OPS.append(op)
    dve_ops.CUSTOM_DVE_SPECS[op.name] = op.spec
    dve_ops._SUB_OPCODE_FOR_NAME[op.name] = (
        dve_ops._CUSTOM_DVE_ROW_BASE + len(dve_ops.OPS) - 1
    )
    return op


RELU_SQ = _register_relu_sq()

B, S, D, H, DH = 4, 2048, 768, 12, 64
NCORES = 8
HL = H // 2          # local heads per core = 6
HPAIRS = HL // 2     # head pairs = 3
DLOC = HL * DH       # local context dims = 384
KTILES = S // 128    # 16
QCHUNK = 512
NQC = S // QCHUNK    # 4
DKT = D // 128       # 6 contraction tiles for projections
OTILES = D // 128    # 6 output-projection row tiles
DHP = 80             # padded per-head V block (DoubleRow needs ko stride %16==0)

F32 = mybir.dt.float32
BF16 = mybir.dt.bfloat16
F16 = mybir.dt.float16
F8 = mybir.dt.float8e4
PM = mybir.MatmulPerfMode
NP_BF16 = ml_dtypes.bfloat16
NP_F16 = np.float16
AF = mybir.ActivationFunctionType
OP = mybir.AluOpType

# AV pair j (k-tiles 2j,2j+1) issues at t-slot 2j + AVP_OFF, so the in-order
# PE stream never waits on the elementwise chain.  Pairs past the block end
# carry into the next block's first slots.
AVP_OFF = 9
# combine micro-ops start at this t-slot of the following block
CB_OFF = 4
# r-AV for k-tile t issues at t-slot t + RV_OFF
RV_OFF = 4
# engine for the outproj psum eviction: "act" | "dve" | "any"
OB_ENGINE = "act"
# engine for the psr psum eviction
XR_ENGINE = "dve"
# number of et/rt pair buffers
EW_BUFS = 6
import os
OUT_DMA = os.environ.get("OUT_DMA", "sp")
# k-tile slots at which outproj o-tiles are emitted (on p != 0 blocks only,
# so the newest ctx q-chunk has a full block of slack before first use)
OUTPROJ_SLOTS = (12, 14, 15)

_KERNEL_CACHE: dict = {}


def build_kernel(m0: float, m1: float, has_bias: bool, repeat: int = 1):
    nc = bacc.Bacc("TRN2", target_bir_lowering=False, debug=False)

    hT = nc.dram_tensor("hT", [D, S], F16, kind="ExternalInput").ap()
    wqT = nc.dram_tensor("wqT", [D, DLOC], F16, kind="ExternalInput").ap()
    wkT = nc.dram_tensor("wkT", [D, DLOC], F16, kind="ExternalInput").ap()
    wvT = nc.dram_tensor("wvT", [D, DLOC], F16, kind="ExternalInput").ap()
    woT = nc.dram_tensor("woT", [DLOC, D], F16, kind="ExternalInput").ap()
    if has_bias:
        hb = nc.dram_tensor("hb", [1, S], F16, kind="ExternalInput").ap()
        wqb = nc.dram_tensor("wqb", [1, DLOC], F16, kind="ExternalInput").ap()
        wkb = nc.dram_tensor("wkb", [1, DLOC], F16, kind="ExternalInput").ap()
        wvb = nc.dram_tensor("wvb", [1, DLOC], F16, kind="ExternalInput").ap()
    # tile-major output: each [128, QCHUNK] store is one contiguous block
    # (the [D, S] layout forced 128 separate 1KB row writes per DMA)
    out = nc.dram_tensor("out", [NQC * OTILES, 128, QCHUNK], F16,
                         kind="ExternalOutput").ap()

    # V is stored unscaled in fp8 (ones column exactly 1.0); the mix factors
    # are applied in the combine: zrec *= m0, and ctx = (xr * m1) + prod.

    with tile.TileContext(nc) as tc, ExitStack() as ctx:
        # ---------------- persistent SBUF ----------------
        pp = ctx.enter_context(tc.tile_pool(name="persist", bufs=1))

        h_t = [pp.tile([128, S], F16, tag=f"ht{k}", name=f"ht{k}") for k in range(DKT)]
        wq_t = [pp.tile([128, DLOC], F16, tag=f"wq{k}", name=f"wq{k}") for k in range(DKT)]
        wk_t = [pp.tile([128, DLOC], F16, tag=f"wk{k}", name=f"wk{k}") for k in range(DKT)]
        wv_t = [pp.tile([128, DLOC], F16, tag=f"wv{k}", name=f"wv{k}") for k in range(DKT)]
        wo_t = [pp.tile([128, D], F16, tag=f"wo{c}", name=f"wo{c}") for c in range(HPAIRS)]
        for k in range(DKT):
            nc.sync.dma_start(wk_t[k][:], wkT[k * 128:(k + 1) * 128, :])
            nc.sync.dma_start(h_t[k][:], hT[k * 128:(k + 1) * 128, :])
        for k in range(DKT):
            nc.sync.dma_start(wq_t[k][:], wqT[k * 128:(k + 1) * 128, :])
        for k in range(DKT):
            nc.sync.dma_start(wv_t[k][:], wvT[k * 128:(k + 1) * 128, :])
        for c in range(HPAIRS):
            nc.sync.dma_start(wo_t[c][:], woT[c * 128:(c + 1) * 128, :])
        if has_bias:
            hb_t = pp.tile([1, S], F16, tag="hbt")
            wqb_t = pp.tile([1, DLOC], F16, tag="wqbt")
            wkb_t = pp.tile([1, DLOC], F16, tag="wkbt")
            wvb_t = pp.tile([1, DLOC], F16, tag="wvbt")
            nc.sync.dma_start(hb_t[:], hb[:, :])
            nc.sync.dma_start(wqb_t[:], wqb[:, :])
            nc.sync.dma_start(wkb_t[:], wkb[:, :])
            nc.sync.dma_start(wvb_t[:], wvb[:, :])

        # Q and K side by side so one ACT copy evicts both per q-chunk
        qk_s = [pp.tile([128, 2 * S], F16, tag=f"qk{p}", name=f"qk{p}") for p in range(HPAIRS)]
        # V pair tiles for fp8 DoubleRow AV: vp_s[j] holds k-tiles 2j (ko=0)
        # and 2j+1 (ko=1); per head DHP cols = [V(64) | ones | pad]
        vp_s = [pp.tile([128, 2 * HL * DHP], F8, tag=f"vp{j}", name=f"vp{j}")
                for j in range(KTILES // 2)]
        # fp16 V copies for the r-branch AV (fp8 V costs ~1.5e-2 rel error)
        vb_s = [pp.tile([128, 2 * DLOC], F16, tag=f"vb{j}", name=f"vb{j}")
                for j in range(KTILES // 2)]
        # per-(p,qc) ctx tiles: avoids false tile-granular dependencies
        # between the combine write of one q-chunk and outproj reads of another
        ctx_s = [[pp.tile([128, QCHUNK], F16, tag=f"cx{p}_{q}", name=f"cx{p}_{q}")
                  for q in range(NQC)] for p in range(HPAIRS)]

        # ones columns are persistent: written once, never overwritten (the V
        # evictions write a strided AP that skips them)
        for j in range(KTILES // 2):
            vp4 = vp_s[j][:, :].rearrange("p (ko a d) -> p ko a d", ko=2, d=DHP)
            nc.gpsimd.memset(vp4[:, :, :, DH:DH + 1], 1.0)

        nkt = DKT + (1 if has_bias else 0)

        def ev_engine(name):
            if name == "act":
                return nc.scalar
            if name == "dve":
                return nc.vector
            return nc.any

        def phases(pend_outproj, outpool, obsb, rep):
            # emit one output-projection o-tile: pso accumulate over head
            # pairs, evict, dma
            def outproj_tile(qc, ot):
                cols = bass.ts(qc, QCHUNK)
                pso = outpool.tile([128, QCHUNK], F32, tag="pso",
                                   name=f"pso_r{rep}_{qc}_{ot}")
                orows = bass.ts(ot, 128)
                for c in range(HPAIRS):
                    nc.tensor.matmul(pso[:], wo_t[c][:, orows],
                                     ctx_s[c][qc][:, :],
                                     start=(c == 0), stop=(c == HPAIRS - 1))
                ob = obsb.tile([128, QCHUNK], F16, tag="ob",
                               name=f"ob_r{rep}_{qc}_{ot}")
                if ot % 2 == 0:
                    nc.scalar.activation(ob[:], pso[:], AF.Copy)
                else:
                    nc.vector.tensor_copy(ob[:], pso[:])
                if OUT_DMA == "gp":
                    nc.gpsimd.dma_start(out[qc * OTILES + ot, :, :], ob[:])
                elif OUT_DMA != "none":
                    nc.sync.dma_start(out[qc * OTILES + ot, :, :], ob[:])

            def drain_outproj(n):
                while len(pend_outproj) > n:
                    qc, ot = pend_outproj.pop(0)
                    outproj_tile(qc, ot)

            # ---------------- phase 2: attention (with fused prefix) ----
            with tc.tile_pool(name="scps", bufs=2, space="PSUM") as scps, \
                 tc.tile_pool(name="acps", bufs=1, space="PSUM") as acps, \
                 tc.tile_pool(name="ewsb", bufs=EW_BUFS) as ewsb, \
                 tc.tile_pool(name="cbsb", bufs=2) as cbsb:

                def q_chain(p_, qc_):
                    # one Q projection chain [128,512] via the aux psum bank
                    ps = outpool.tile([128, QCHUNK], F32, tag="pso", name="qch")
                    for k in range(nkt):
                        rhs = h_t[k][:, bass.ts(qc_, QCHUNK)] if k < DKT \
                            else hb_t[:, bass.ts(qc_, QCHUNK)]
                        wl = wq_t[k][:, p_ * 128:(p_ + 1) * 128] if k < DKT \
                            else wqb_t[:, p_ * 128:(p_ + 1) * 128]
                        nc.tensor.matmul(ps[:], wl, rhs, start=(k == 0),
                                         stop=(k == nkt - 1))
                    nc.scalar.activation(qk_s[p_][:, qc_ * QCHUNK:(qc_ + 1) * QCHUNK],
                                         ps[:], AF.Copy)

                # ---- prefix: all K, Q(q0), all V (scores need full K; AVs
                # need V; Q(qc>0) chains are interleaved into earlier blocks)
                drain_outproj(0)
                for p_ in range(HPAIRS):
                    for g in range(2):
                        ps = scps.tile([128, 2 * QCHUNK], F32, tag="s", name="kpre")
                        for half in range(2):
                            kc = 2 * g + half
                            dst = slice(half * QCHUNK, (half + 1) * QCHUNK)
                            for k in range(nkt):
                                rhs = h_t[k][:, bass.ts(kc, QCHUNK)] if k < DKT \
                                    else hb_t[:, bass.ts(kc, QCHUNK)]
                                wl = wk_t[k][:, p_ * 128:(p_ + 1) * 128] if k < DKT \
                                    else wkb_t[:, p_ * 128:(p_ + 1) * 128]
                                nc.tensor.matmul(ps[:, dst], wl, rhs, start=(k == 0),
                                                 stop=(k == nkt - 1))
                        nc.scalar.activation(
                            qk_s[p_][:, S + 2 * g * QCHUNK:S + 2 * (g + 1) * QCHUNK],
                            ps[:], AF.Copy)
                for p_ in range(HPAIRS):
                    q_chain(p_, 0)
                for g in range(KTILES // 2):
                    ps = scps.tile([128, 2 * QCHUNK], F32, tag="s", name="vpre")
                    vp4 = vp_s[g][:, :].rearrange("p (ko a d) -> p ko a d",
                                                  ko=2, d=DHP)
                    for half in range(2):
                        t_ = 2 * g + half
                        vdst = slice(half * QCHUNK, half * QCHUNK + DLOC)
                        for k in range(nkt):
                            lhsT = h_t[k][:, bass.ts(t_, 128)] if k < DKT \
                                else hb_t[:, bass.ts(t_, 128)]
                            rhs = wv_t[k][:] if k < DKT else wvb_t[:]
                            nc.tensor.matmul(ps[:, vdst], lhsT, rhs, start=(k == 0),
                                             stop=(k == nkt - 1))
                        psv_4d = ps[:, vdst].rearrange("p (ko a d) -> p ko a d",
                                                       ko=1, d=DH)
                        nc.scalar.activation(vp4[:, half:half + 1, :, 0:DH],
                                             psv_4d[:, :, :, :], AF.Copy)
                        vb3 = vb_s[g][:, :].rearrange("p (ko x) -> p ko x", ko=2)
                        nc.scalar.activation(vb3[:, half:half + 1, :], 
                                             ps[:, vdst].rearrange("p (ko x) -> p ko x", ko=1),
                                             AF.Copy)

                def cb_step(cb, step):
                    """One micro-op of the deferred combine for the previous
                    block; spread across the next block's k-tile loop so the
                    ACT/DVE FIFOs never see a burst."""
                    if cb is None:
                        return
                    p_, pse_a, pse_b, psr_, cols_ = (
                        cb["p"], cb["pse_a"], cb["pse_b"], cb["psr"], cb["cols"])
                    if step == 0:
                        # fold the relu^2 mix weight m1 into the psr eviction
                        cb["xr"] = cbsb.tile([128, QCHUNK], F32, tag="xr", name="xr")
                        if XR_ENGINE == "act":
                            nc.scalar.activation(cb["xr"][:], psr_[:], AF.Copy, scale=m1)
                        else:
                            nc.vector.tensor_scalar_mul(cb["xr"][:], psr_[:], m1)
                    elif step == 1:
                        # fold the softmax mix weight m0 into the pse eviction
                        cb["exq"] = cbsb.tile([128, QCHUNK], F32, tag="exq", name="exq")
                        nc.scalar.activation(cb["exq"][0:64, :], pse_a[0:64, :], AF.Copy,
                                             scale=m0)
                    elif step == 2:
                        cb["zrow_a"] = cbsb.tile([1, QCHUNK], F32, tag="zwa", name="zwa")
                        nc.scalar.activation(cb["zrow_a"][0:1, :], pse_a[64:65, :], AF.Copy)
                    elif step == 3:
                        nc.scalar.activation(cb["exq"][64:128, :], pse_b[0:64, :], AF.Copy,
                                             scale=m0)
                    elif step == 4:
                        cb["zrow_b"] = cbsb.tile([1, QCHUNK], F32, tag="zwb", name="zwb")
                        nc.scalar.activation(cb["zrow_b"][0:1, :], pse_b[64:65, :], AF.Copy)
                    elif step == 5:
                        cb["zrec_a"] = cbsb.tile([1, QCHUNK], F32, tag="zra", name="zra")
                        cb["zrec_b"] = cbsb.tile([1, QCHUNK], F32, tag="zrb", name="zrb")
                        nc.vector.reciprocal_approx_fast(cb["zrec_a"][:], cb["zrow_a"][:])
                        nc.vector.reciprocal_approx_fast(cb["zrec_b"][:], cb["zrow_b"][:])
                    elif step == 7:
                        cb["zb1"] = cbsb.tile([128, QCHUNK], F32, tag="zb1", name="zb1")
                        nc.gpsimd.partition_broadcast(cb["zb1"][:, :], cb["zrec_a"][0:1, :],
                                                      channels=128)
                    elif step == 8:
                        cb["zb2"] = cbsb.tile([128, QCHUNK], F32, tag="zb2", name="zb2")
                        nc.gpsimd.partition_broadcast(cb["zb2"][:, :], cb["zrec_b"][0:1, :],
                                                      channels=128)
                    elif step == 9:
                        cb["prod"] = cbsb.tile([128, QCHUNK], F32, tag="prod", name="prod")
                        nc.gpsimd.tensor_tensor(cb["prod"][0:64, :], cb["exq"][0:64, :],
                                                cb["zb1"][0:64, :], op=OP.mult)
                    elif step == 10:
                        nc.gpsimd.tensor_tensor(cb["prod"][64:128, :], cb["exq"][64:128, :],
                                                cb["zb2"][64:128, :], op=OP.mult)
                    elif step == 11:
                        nc.gpsimd.tensor_tensor(ctx_s[p_][cb["qc"]][:, :], cb["prod"][:],
                                                cb["xr"][:], op=OP.add)
                        cb["done"] = True

                N_CB_STEPS = 12
                pending_cb = None
                pending_av = None

                for qc in range(NQC):
                    for p in range(HPAIRS):
                        a0, a1 = 2 * p, 2 * p + 1
                        cols = bass.ts(qc, QCHUNK)
                        pse_a = acps.tile([128, QCHUNK], F32, tag="peA")
                        pse_b = acps.tile([128, QCHUNK], F32, tag="peB")
                        psr = acps.tile([128, QCHUNK], F32, tag="pr")
                        pending = {}
                        pending_r = {}
                        blk = {"pse_a": pse_a, "pse_b": pse_b, "psr": psr,
                               "a0": a0, "a1": a1, "pending": pending,
                               "pending_r": pending_r}

                        def av_e_for(bk, j):
                            # fp8 DoubleRow e-AV over k-tile pair (2j, 2j+1)
                            ep = bk["pending"].pop(j)
                            st, sp = j == 0, j == KTILES // 2 - 1
                            b0, b1 = bk["a0"], bk["a1"]
                            e3 = ep[:, :].rearrange("p (ko x) -> p ko x", ko=2)
                            v3 = vp_s[j][:, :].rearrange("p (ko x) -> p ko x", ko=2)
                            va = v3[:, :, b0 * DHP:b0 * DHP + DH + 1]
                            vb = v3[:, :, b1 * DHP:b1 * DHP + DH + 1]
                            nc.tensor.matmul(bk["pse_a"][0:DH + 1, :], va, e3[:, :, 0:QCHUNK],
                                             start=st, stop=sp, perf_mode=PM.DoubleRow)
                            nc.tensor.matmul(bk["pse_b"][0:DH + 1, :], vb,
                                             e3[:, :, QCHUNK:2 * QCHUNK],
                                             start=st, stop=sp, perf_mode=PM.DoubleRow)

                        def av_r_for(bk, t):
                            # bf16 r-AV for k-tile t (col-packed head pair)
                            rt = bk["pending_r"].pop(t)
                            st, sp = t == 0, t == KTILES - 1
                            b0, b1 = bk["a0"], bk["a1"]
                            j, ph = t // 2, t % 2
                            v3 = vb_s[j][:, :].rearrange("p (ko x) -> p ko x", ko=2)
                            nc.tensor.matmul(bk["psr"][0:64, :],
                                             v3[:, ph:ph + 1, b0 * DH:(b0 + 1) * DH],
                                             rt[:, 0:QCHUNK], start=st, stop=sp)
                            nc.tensor.matmul(bk["psr"][64:128, :],
                                             v3[:, ph:ph + 1, b1 * DH:(b1 + 1) * DH],
                                             rt[:, QCHUNK:2 * QCHUNK], start=st, stop=sp)

                        for t in range(KTILES):
                            kcols = slice(S + t * 128, S + (t + 1) * 128)
                            qcols = slice(qc * QCHUNK, (qc + 1) * QCHUNK)
                            # both heads' score tiles side by side in one 2-bank
                            # PSUM tile; the two matmuls row-pack (tile_position
                            # (0,0) and (64,0) via base partitions)
                            ss = scps.tile([128, 2 * QCHUNK], F32, tag="s")
                            nc.tensor.matmul(ss[:, 0:QCHUNK], qk_s[p][0:64, kcols],
                                             qk_s[p][0:64, qcols])
                            nc.tensor.matmul(ss[:, QCHUNK:2 * QCHUNK], qk_s[p][64:128, kcols],
                                             qk_s[p][64:128, qcols])

                            # e pair tiles fp8 (ko-major halves); r tiles bf16
                            if t % 2 == 0:
                                etp = ewsb.tile([128, 4 * QCHUNK], F8, tag="e", name="e")
                                pending[t // 2] = etp
                            half = slice((t % 2) * 2 * QCHUNK, (t % 2 + 1) * 2 * QCHUNK)
                            rt = ewsb.tile([128, 2 * QCHUNK], F16, tag="r", name="r")
                            pending_r[t] = rt
                            nc.scalar.activation(etp[:, half], ss[:], AF.Exp)
                            nc.vector._custom_dve(RELU_SQ, out=rt[:], in0=ss[:])
                            # previous block's tail AVs, spread over early slots
                            if pending_av is not None:
                                bk_p, etail, rtail = pending_av
                                if t < len(etail):
                                    av_e_for(bk_p, etail[t])
                                if t < len(rtail):
                                    av_r_for(bk_p, rtail[t])
                            # previous block's combine, one micro-op per tile
                            cb_step(pending_cb, t - CB_OFF)
                            if t >= AVP_OFF and (t - AVP_OFF) % 2 == 0:
                                av_e_for(blk, (t - AVP_OFF) // 2)
                            if t >= RV_OFF:
                                av_r_for(blk, t - RV_OFF)
                            # next q-chunk's Q projection, one chain per block
                            if t == 5 and qc + 1 < NQC:
                                q_chain(p, qc + 1)
                            # pending outproj o-tiles mid-block
                            if p != 0 and t in OUTPROJ_SLOTS and pend_outproj:
                                qc_o, ot_o = pend_outproj.pop(0)
                                outproj_tile(qc_o, ot_o)
                        pending_av_next = (blk, sorted(pending), sorted(pending_r))

                        # defer this block's tail AVs and combine into the
                        # next block's loop
                        pending_av = pending_av_next
                        pending_cb = {"p": p, "pse_a": pse_a, "pse_b": pse_b,
                                      "psr": psr, "cols": cols, "qc": qc}

                        # queue this q-chunk's output projection once all head
                        # pairs are done; emitted interleaved in later blocks
                        if p == HPAIRS - 1:
                            for ot in range(OTILES):
                                pend_outproj.append((qc, ot))

                # drain the last block's tail AVs and combine
                if pending_av is not None:
                    for j in pending_av[1]:
                        av_e_for(pending_av[0], j)
                    for tt in pending_av[2]:
                        av_r_for(pending_av[0], tt)
                for st_i in range(N_CB_STEPS):
                    cb_step(pending_cb, st_i)

        pend_outproj: list = []
        with tc.tile_pool(name="outps", bufs=1, space="PSUM") as outpool, \
             tc.tile_pool(name="obsb", bufs=2) as obsb:
            for _rep in range(repeat):
                phases(pend_outproj, outpool, obsb, _rep)
            # tail: remaining outproj tiles of the last rep (phase pools are
            # closed here, so banks are free for a wider tail pool)
            with tc.tile_pool(name="tailps", bufs=3, space="PSUM") as tailpool:
                while pend_outproj:
                    qc, ot = pend_outproj.pop(0)
                    cols = bass.ts(qc, QCHUNK)
                    pso = tailpool.tile([128, QCHUNK], F32, tag="pso",
                                        name=f"pso_tail_{qc}_{ot}")
                    orows = bass.ts(ot, 128)
                    for c in range(HPAIRS):
                        nc.tensor.matmul(pso[:], wo_t[c][:, orows],
                                         ctx_s[c][qc][:, :],
                                         start=(c == 0), stop=(c == HPAIRS - 1))
                    ob = obsb.tile([128, QCHUNK], F16, tag="ob",
                                   name=f"ob_tail_{qc}_{ot}")
                    if ot % 2 == 0:
                        nc.scalar.activation(ob[:], pso[:], AF.Copy)
                    else:
                        nc.vector.tensor_copy(ob[:], pso[:])
                    if OUT_DMA == "gp":
                        nc.gpsimd.dma_start(out[qc * OTILES + ot, :, :], ob[:])
                    elif OUT_DMA != "none":
                        nc.sync.dma_start(out[qc * OTILES + ot, :, :], ob[:])

    nc.compile()
    return nc


def _get_kernel(m0: float, m1: float, has_bias: bool):
    key = (round(m0, 9), round(m1, 9), has_bias)
    if key not in _KERNEL_CACHE:
        _KERNEL_CACHE[key] = build_kernel(m0, m1, has_bias)
    return _KERNEL_CACHE[key]


def make_in_maps(inputs: dict) -> tuple[list[dict], float, float, bool]:
    hidden = np.asarray(inputs["hidden_states"], dtype=np.float32)
    Wq = np.asarray(inputs["Wq"], dtype=np.float32)
    Wk = np.asarray(inputs["Wk"], dtype=np.float32)
    Wv = np.asarray(inputs["Wv"], dtype=np.float32)
    Wo = np.asarray(inputs["Wo"], dtype=np.float32)
    bq = np.asarray(inputs["bq"], dtype=np.float32)
    bk = np.asarray(inputs["bk"], dtype=np.float32)
    bv = np.asarray(inputs["bv"], dtype=np.float32)
    w_mix = np.asarray(inputs["w_mix"], dtype=np.float32)

    e = np.exp(w_mix - w_mix.max())
    mix = e / e.sum()
    m0, m1 = float(mix[0]), float(mix[1])
    has_bias = bool(bq.any() or bk.any() or bv.any())

    qk_scale = 1.0 / float(np.sqrt(DH))

    def bf(x):
        return np.ascontiguousarray(x).astype(NP_F16)

    in_maps = []
    for core in range(NCORES):
        b, g = core // 2, core % 2
        rows = slice(DLOC * g, DLOC * (g + 1))
        m = {
            "hT": bf(hidden[b].T),
            "wqT": bf(Wq[rows].T * qk_scale),
            "wkT": bf(Wk[rows].T),
            "wvT": bf(Wv[rows].T),
            "woT": bf(Wo[:, rows].T),
        }
        if has_bias:
            m["hb"] = bf(np.ones((1, S), dtype=np.float32))
            m["wqb"] = bf(bq[rows][None, :] * qk_scale)
            m["wkb"] = bf(bk[rows][None, :])
            m["wvb"] = bf(bv[rows][None, :])
        in_maps.append(m)
    return in_maps, m0, m1, has_bias


def assemble_output(results: list[dict], bo: np.ndarray) -> np.ndarray:
    out = np.empty((B, S, D), dtype=np.float32)
    for b in range(B):
        # [NQC*OTILES, 128, QCHUNK] tile-major -> [D, S]
        acc = (results[2 * b]["out"].astype(np.float32) +
               results[2 * b + 1]["out"].astype(np.float32))
        full = acc.reshape(NQC, OTILES, 128, QCHUNK).transpose(1, 2, 0, 3) \
                  .reshape(D, S)
        out[b] = full.T
    if bo.any():
        out += bo
    return out


def _spot_check(out: np.ndarray, inputs: dict, rng: np.random.Generator) -> bool:
    """Recompute one random query row per batch on the host (covers all 8
    cores' partial outputs) and compare; guards against transient HW faults."""
    hidden = np.asarray(inputs["hidden_states"], dtype=np.float32)
    Wq = np.asarray(inputs["Wq"], dtype=np.float32)
    Wk = np.asarray(inputs["Wk"], dtype=np.float32)
    Wv = np.asarray(inputs["Wv"], dtype=np.float32)
    Wo = np.asarray(inputs["Wo"], dtype=np.float32)
    bq = np.asarray(inputs["bq"], dtype=np.float32)
    bk = np.asarray(inputs["bk"], dtype=np.float32)
    bv = np.asarray(inputs["bv"], dtype=np.float32)
    bo = np.asarray(inputs["bo"], dtype=np.float32)
    w_mix = np.asarray(inputs["w_mix"], dtype=np.float32)
    e = np.exp(w_mix - w_mix.max())
    m0, m1 = e / e.sum()
    for b in range(B):
        s = int(rng.integers(0, S))
        q = (hidden[b, s] @ Wq.T + bq).reshape(H, DH) / np.sqrt(DH)
        k = (hidden[b] @ Wk.T + bk).reshape(S, H, DH)
        v = (hidden[b] @ Wv.T + bv).reshape(S, H, DH)
        scores = np.einsum("hd,khd->hk", q, k)
        sm = np.exp(scores - scores.max(axis=1, keepdims=True))
        sm /= sm.sum(axis=1, keepdims=True)
        attn = m0 * sm + m1 * np.maximum(scores, 0.0) ** 2
        ctx = np.einsum("hk,khd->hd", attn, v).reshape(D)
        want = ctx @ Wo.T + bo
        got = out[b, s]
        rel = np.abs(got - want).max() / max(np.abs(want).max(), 1e-6)
        if not np.isfinite(got).all() or rel > 0.05:
            return False
    return True


def kernel(**inputs) -> np.ndarray:
    in_maps, m0, m1, has_bias = make_in_maps(inputs)
    nc = _get_kernel(m0, m1, has_bias)
    bo = np.asarray(inputs["bo"], dtype=np.float32)
    rng = np.random.default_rng(12345)
    out = None
    for _attempt in range(3):
        res = run_bass_kernel_spmd(nc, in_maps, core_ids=list(range(NCORES)))
        out = assemble_output(res.results, bo)
        if np.isfinite(out).all() and _spot_check(out, inputs, rng):
            return out
    return out


# revision 36
# speedup vs baseline: 2.4098x; 1.5664x over previous
"""Trainium2 Bass kernel for mixed softmax + relu^2 attention (v2).

Reference computation (B=4, S=2048, D=768, H=12, DH=64):
    q = split_heads(hidden @ Wq.T + bq)        # [B,H,S,DH]
    k = split_heads(hidden @ Wk.T + bk)
    v = split_heads(hidden @ Wv.T + bv)
    scores = q @ k.T / sqrt(DH)                # [B,H,S,S]
    attn = m0 * softmax(scores) + m1 * relu(scores)^2,  (m0,m1) = softmax(w_mix)
    out = merge_heads(attn @ v) @ Wo.T + bo

Sharding over 8 NeuronCores: core = (batch b = core//2, head-group g = core%2 of
6 heads).  Each core computes its 6 heads' full SxS attention and a partial
output projection over its 384 context dims; the host sums the two partials
per batch.

Device-side layout ("transposed", k on partitions), per head pair p (2 heads
a0/a1 stacked on partitions 0-63 / 64-127):
  - qk[p] [128, 2S]: Q cols [0,S) (pre-scaled by 1/sqrt(DH) via host-side
    Wq scaling), K cols [S,2S).  Head-major rows.  Evicted from a single
    2-bank PSUM tile with one ACT copy per q-chunk.
  - scoresT tile ss [k=128, 2*512] = K_tile.T @ Q_chunk for both heads
    (row-packed concurrent matmuls via auto tile_position).
  - e = exp(ss) on ACT -> bf16; r = relu(ss)^2 on DVE (custom op) -> bf16.
  - V augmented per head: [alpha*V | beta] where alpha=max(m1,eps),
    beta=alpha/m0; e-AV accumulates [alpha*V|beta].T @ e so row 64 holds
    beta*Z (Z = softmax denominator); r-AV accumulates (alpha*V).T @ r
    col-packed for both heads into one psum tile.
  - combine: ACT evicts pse rows 0-64 -> SBUF; DVE reciprocal of the
    beta*Z rows (PSUM); GpSimd broadcasts 1/(beta*Z), multiplies and adds:
    ctx = ex * zb + xr  (equals m0*V.T e/Z + m1*V.T r by construction).
  - out_partial[o, s] = Wo_part.T @ ctx per 128-row o-tile, interleaved one
    o-tile per k-tile iteration of a later block; shipped fp32; host sums.
"""

from contextlib import ExitStack

import numpy as np
import ml_dtypes

import concourse.bass as bass
import concourse.mybir as mybir
import concourse.tile as tile
from concourse import bacc, dve_ops
from concourse.bass_utils import run_bass_kernel_spmd
from concourse.dve_spec import Spec, Src0, relu as _sp_relu, sq as _sp_sq


def _register_relu_sq():
    """Custom fused DVE op: out = relu(in0)^2 in a single pass."""
    for op in dve_ops.OPS:
        if op.name == "RELU_SQ_ANT":
            return op
    op = dve_ops.DveOp(
        "RELU_SQ_ANT",
        Spec(body=_sp_sq(_sp_relu(Src0)),
             reference=lambda in0: np.maximum(in0, 0.0) ** 2),
        subdim=False,
        uops_sha={"v3": "8abca05ebc329c1b", "v4": "4b83c053374efcdc"},
    )
    dve_ops.OPS.append(op)
    dve_ops.CUSTOM_DVE_SPECS[op.name] = op.spec
    dve_ops._SUB_OPCODE_FOR_NAME[op.name] = (
        dve_ops._CUSTOM_DVE_ROW_BASE + len(dve_ops.OPS) - 1
    )
    return op


RELU_SQ = _register_relu_sq()

B, S, D, H, DH = 4, 2048, 768, 12, 64
NCORES = 8
HL = H // 2          # local heads per core = 6
HPAIRS = HL // 2     # head pairs = 3
DLOC = HL * DH       # local context dims = 384
KTILES = S // 128    # 16
QCHUNK = 512
NQC = S // QCHUNK    # 4
DKT = D // 128       # 6 contraction tiles for projections
OTILES = D // 128    # 6 output-projection row tiles
DHP = 80             # padded per-head V block (DoubleRow needs ko stride %16==0)

F32 = mybir.dt.float32
BF16 = mybir.dt.bfloat16
F16 = mybir.dt.float16
F8 = mybir.dt.float8e4
PM = mybir.MatmulPerfMode
NP_BF16 = ml_dtypes.bfloat16
NP_F16 = np.float16
AF = mybir.ActivationFunctionType
OP = mybir.AluOpType

# AV pair j (k-tiles 2j,2j+1) issues at t-slot 2j + AVP_OFF, so the in-order
# PE stream never waits on the elementwise chain.  Pairs past the block end
# carry into the next block's first slots.
AVP_OFF = 9
# combine micro-ops start at this t-slot of the following block
CB_OFF = 4
# r-AV for k-tile t issues at t-slot t + RV_OFF
RV_OFF = 4
# engine for the outproj psum eviction: "act" | "dve" | "any"
OB_ENGINE = "act"
# engine for the psr psum eviction
XR_ENGINE = "dve"
# number of et/rt pair buffers
EW_BUFS = 6
import os
OUT_DMA = os.environ.get("OUT_DMA", "sp")
# k-tile slots at which outproj o-tiles are emitted (on p != 0 blocks only,
# so the newest ctx q-chunk has a full block of slack before first use)
OUTPROJ_SLOTS = (12, 14, 15)

_KERNEL_CACHE: dict = {}


def build_kernel(m0: float, m1: float, has_bias: bool, repeat: int = 1):
    nc = bacc.Bacc("TRN2", target_bir_lowering=False, debug=False)

    hT = nc.dram_tensor("hT", [D, S], F16, kind="ExternalInput").ap()
    wqT = nc.dram_tensor("wqT", [D, DLOC], F16, kind="ExternalInput").ap()
    wkT = nc.dram_tensor("wkT", [D, DLOC], F16, kind="ExternalInput").ap()
    wvT = nc.dram_tensor("wvT", [D, DLOC], F16, kind="ExternalInput").ap()
    woT = nc.dram_tensor("woT", [DLOC, D], F16, kind="ExternalInput").ap()
    if has_bias:
        hb = nc.dram_tensor("hb", [1, S], F16, kind="ExternalInput").ap()
        wqb = nc.dram_tensor("wqb", [1, DLOC], F16, kind="ExternalInput").ap()
        wkb = nc.dram_tensor("wkb", [1, DLOC], F16, kind="ExternalInput").ap()
        wvb = nc.dram_tensor("wvb", [1, DLOC], F16, kind="ExternalInput").ap()
    # tile-major output: each [128, QCHUNK] store is one contiguous block
    # (the [D, S] layout forced 128 separate 1KB row writes per DMA)
    out = nc.dram_tensor("out", [NQC * OTILES, 128, QCHUNK], F16,
                         kind="ExternalOutput").ap()

    # V is stored unscaled in fp8 (ones column exactly 1.0); the mix factors
    # are applied in the combine: zrec *= m0, and ctx = (xr * m1) + prod.

    with tile.TileContext(nc) as tc, ExitStack() as ctx:
        # ---------------- persistent SBUF ----------------
        pp = ctx.enter_context(tc.tile_pool(name="persist", bufs=1))

        h_t = [pp.tile([128, S], F16, tag=f"ht{k}", name=f"ht{k}") for k in range(DKT)]
        wq_t = [pp.tile([128, DLOC], F16, tag=f"wq{k}", name=f"wq{k}") for k in range(DKT)]
        wk_t = [pp.tile([128, DLOC], F16, tag=f"wk{k}", name=f"wk{k}") for k in range(DKT)]
        wv_t = [pp.tile([128, DLOC], F16, tag=f"wv{k}", name=f"wv{k}") for k in range(DKT)]
        wo_t = [pp.tile([128, D], F16, tag=f"wo{c}", name=f"wo{c}") for c in range(HPAIRS)]
        for k in range(DKT):
            nc.sync.dma_start(wk_t[k][:], wkT[k * 128:(k + 1) * 128, :])
            nc.sync.dma_start(h_t[k][:], hT[k * 128:(k + 1) * 128, :])
        for k in range(DKT):
            nc.sync.dma_start(wq_t[k][:], wqT[k * 128:(k + 1) * 128, :])
        for k in range(DKT):
            nc.sync.dma_start(wv_t[k][:], wvT[k * 128:(k + 1) * 128, :])
        for c in range(HPAIRS):
            nc.sync.dma_start(wo_t[c][:], woT[c * 128:(c + 1) * 128, :])
        if has_bias:
            hb_t = pp.tile([1, S], F16, tag="hbt")
            wqb_t = pp.tile([1, DLOC], F16, tag="wqbt")
            wkb_t = pp.tile([1, DLOC], F16, tag="wkbt")
            wvb_t = pp.tile([1, DLOC], F16, tag="wvbt")
            nc.sync.dma_start(hb_t[:], hb[:, :])
            nc.sync.dma_start(wqb_t[:], wqb[:, :])
            nc.sync.dma_start(wkb_t[:], wkb[:, :])
            nc.sync.dma_start(wvb_t[:], wvb[:, :])

        # Q and K side by side so one ACT copy evicts both per q-chunk
        qk_s = [pp.tile([128, 2 * S], F16, tag=f"qk{p}", name=f"qk{p}") for p in range(HPAIRS)]
        # V pair tiles for fp8 DoubleRow AV: vp_s[j] holds k-tiles 2j (ko=0)
        # and 2j+1 (ko=1); per head DHP cols = [V(64) | ones | pad]
        vp_s = [pp.tile([128, 2 * HL * DHP], F8, tag=f"vp{j}", name=f"vp{j}")
                for j in range(KTILES // 2)]
        # fp16 V copies for the r-branch AV (fp8 V costs ~1.5e-2 rel error)
        vb_s = [pp.tile([128, 2 * DLOC], F16, tag=f"vb{j}", name=f"vb{j}")
                for j in range(KTILES // 2)]
        # per-(p,qc) ctx tiles: avoids false tile-granular dependencies
        # between the combine write of one q-chunk and outproj reads of another
        ctx_s = [[pp.tile([128, QCHUNK], F16, tag=f"cx{p}_{q}", name=f"cx{p}_{q}")
                  for q in range(NQC)] for p in range(HPAIRS)]

        # ones columns are persistent: written once, never overwritten (the V
        # evictions write a strided AP that skips them)
        for j in range(KTILES // 2):
            vp4 = vp_s[j][:, :].rearrange("p (ko a d) -> p ko a d", ko=2, d=DHP)
            nc.gpsimd.memset(vp4[:, :, :, DH:DH + 1], 1.0)

        nkt = DKT + (1 if has_bias else 0)

        def ev_engine(name):
            if name == "act":
                return nc.scalar
            if name == "dve":
                return nc.vector
            return nc.any

        def phases(pend_outproj, outpool, obsb, rep):
            # emit one output-projection o-tile: pso accumulate over head
            # pairs, evict, dma
            def outproj_tile(qc, ot):
                cols = bass.ts(qc, QCHUNK)
                pso = outpool.tile([128, QCHUNK], F32, tag="pso",
                                   name=f"pso_r{rep}_{qc}_{ot}")
                orows = bass.ts(ot, 128)
                for c in range(HPAIRS):
                    nc.tensor.matmul(pso[:], wo_t[c][:, orows],
                                     ctx_s[c][qc][:, :],
                                     start=(c == 0), stop=(c == HPAIRS - 1))
                ob = obsb.tile([128, QCHUNK], F16, tag="ob",
                               name=f"ob_r{rep}_{qc}_{ot}")
                if ot % 2 == 0:
                    nc.scalar.activation(ob[:], pso[:], AF.Copy)
                else:
                    nc.vector.tensor_copy(ob[:], pso[:])
                if OUT_DMA == "gp":
                    nc.gpsimd.dma_start(out[qc * OTILES + ot, :, :], ob[:])
                elif OUT_DMA != "none":
                    nc.sync.dma_start(out[qc * OTILES + ot, :, :], ob[:])

            def drain_outproj(n):
                while len(pend_outproj) > n:
                    qc, ot = pend_outproj.pop(0)
                    outproj_tile(qc, ot)

            # ---------------- phase 2: attention (with fused prefix) ----
            with tc.tile_pool(name="scps", bufs=2, space="PSUM") as scps, \
                 tc.tile_pool(name="acps", bufs=1, space="PSUM") as acps, \
                 tc.tile_pool(name="ewsb", bufs=EW_BUFS) as ewsb, \
                 tc.tile_pool(name="cbsb", bufs=2) as cbsb:

                def q_chain(p_, qc_):
                    # one Q projection chain [128,512] via the aux psum bank
                    ps = outpool.tile([128, QCHUNK], F32, tag="pso", name="qch")
                    for k in range(nkt):
                        rhs = h_t[k][:, bass.ts(qc_, QCHUNK)] if k < DKT \
                            else hb_t[:, bass.ts(qc_, QCHUNK)]
                        wl = wq_t[k][:, p_ * 128:(p_ + 1) * 128] if k < DKT \
                            else wqb_t[:, p_ * 128:(p_ + 1) * 128]
                        nc.tensor.matmul(ps[:], wl, rhs, start=(k == 0),
                                         stop=(k == nkt - 1))
                    nc.scalar.activation(qk_s[p_][:, qc_ * QCHUNK:(qc_ + 1) * QCHUNK],
                                         ps[:], AF.Copy)

                # ---- prefix: all K, Q(q0), all V (scores need full K; AVs
                # need V; Q(qc>0) chains are interleaved into earlier blocks)
                drain_outproj(0)
                for p_ in range(HPAIRS):
                    for g in range(2):
                        ps = scps.tile([128, 2 * QCHUNK], F32, tag="s", name="kpre")
                        for half in range(2):
                            kc = 2 * g + half
                            dst = slice(half * QCHUNK, (half + 1) * QCHUNK)
                            for k in range(nkt):
                                rhs = h_t[k][:, bass.ts(kc, QCHUNK)] if k < DKT \
                                    else hb_t[:, bass.ts(kc, QCHUNK)]
                                wl = wk_t[k][:, p_ * 128:(p_ + 1) * 128] if k < DKT \
                                    else wkb_t[:, p_ * 128:(p_ + 1) * 128]
                                nc.tensor.matmul(ps[:, dst], wl, rhs, start=(k == 0),
                                                 stop=(k == nkt - 1))
                        nc.scalar.activation(
                            qk_s[p_][:, S + 2 * g * QCHUNK:S + 2 * (g + 1) * QCHUNK],
                            ps[:], AF.Copy)
                for p_ in range(HPAIRS):
                    q_chain(p_, 0)
                for g in range(KTILES // 2):
                    ps = scps.tile([128, 2 * QCHUNK], F32, tag="s", name="vpre")
                    vp4 = vp_s[g][:, :].rearrange("p (ko a d) -> p ko a d",
                                                  ko=2, d=DHP)
                    for half in range(2):
                        t_ = 2 * g + half
                        vdst = slice(half * QCHUNK, half * QCHUNK + DLOC)
                        for k in range(nkt):
                            lhsT = h_t[k][:, bass.ts(t_, 128)] if k < DKT \
                                else hb_t[:, bass.ts(t_, 128)]
                            rhs = wv_t[k][:] if k < DKT else wvb_t[:]
                            nc.tensor.matmul(ps[:, vdst], lhsT, rhs, start=(k == 0),
                                             stop=(k == nkt - 1))
                        psv_4d = ps[:, vdst].rearrange("p (ko a d) -> p ko a d",
                                                       ko=1, d=DH)
                        nc.scalar.activation(vp4[:, half:half + 1, :, 0:DH],
                                             psv_4d[:, :, :, :], AF.Copy)
                        vb3 = vb_s[g][:, :].rearrange("p (ko x) -> p ko x", ko=2)
                        nc.scalar.activation(vb3[:, half:half + 1, :], 
                                             ps[:, vdst].rearrange("p (ko x) -> p ko x", ko=1),
                                             AF.Copy)

                def cb_step(cb, step):
                    """One micro-op of the deferred combine for the previous
                    block; spread across the next block's k-tile loop so the
                    ACT/DVE FIFOs never see a burst."""
                    if cb is None:
                        return
                    p_, pse, psr_, cols_ = (
                        cb["p"], cb["pse"], cb["psr"], cb["cols"])
                    if step == 0:
                        # fold the relu^2 mix weight m1 into the psr eviction
                        cb["xr"] = cbsb.tile([128, QCHUNK], F32, tag="xr", name="xr")
                        if XR_ENGINE == "act":
                            nc.scalar.activation(cb["xr"][:], psr_[:], AF.Copy, scale=m1)
                        else:
                            nc.vector.tensor_scalar_mul(cb["xr"][:], psr_[:], m1)
                    elif step == 1:
                        # fold the softmax mix weight m0 into the pse eviction
                        cb["exq"] = cbsb.tile([128, QCHUNK], F32, tag="exq", name="exq")
                        nc.scalar.activation(cb["exq"][0:64, :], pse[0:64, 0:QCHUNK],
                                             AF.Copy, scale=m0)
                    elif step == 2:
                        # both heads' beta*Z rows in one op (pse halves adjacent)
                        cb["zrow"] = cbsb.tile([1, 2 * QCHUNK], F32, tag="zw", name="zw")
                        nc.scalar.activation(cb["zrow"][0:1, :], pse[64:65, :], AF.Copy)
                    elif step == 3:
                        nc.scalar.activation(cb["exq"][64:128, :],
                                             pse[0:64, QCHUNK:2 * QCHUNK],
                                             AF.Copy, scale=m0)
                    elif step == 5:
                        cb["zrec"] = cbsb.tile([1, 2 * QCHUNK], F32, tag="zr", name="zr")
                        nc.vector.reciprocal_approx_fast(cb["zrec"][:], cb["zrow"][:])
                    elif step == 7:
                        cb["zb1"] = cbsb.tile([128, QCHUNK], F32, tag="zb1", name="zb1")
                        nc.gpsimd.partition_broadcast(cb["zb1"][:, :],
                                                      cb["zrec"][0:1, 0:QCHUNK],
                                                      channels=128)
                    elif step == 8:
                        cb["zb2"] = cbsb.tile([128, QCHUNK], F32, tag="zb2", name="zb2")
                        nc.gpsimd.partition_broadcast(cb["zb2"][:, :],
                                                      cb["zrec"][0:1, QCHUNK:2 * QCHUNK],
                                                      channels=128)
                    elif step == 9:
                        cb["prod"] = cbsb.tile([128, QCHUNK], F32, tag="prod", name="prod")
                        nc.gpsimd.tensor_tensor(cb["prod"][0:64, :], cb["exq"][0:64, :],
                                                cb["zb1"][0:64, :], op=OP.mult)
                    elif step == 10:
                        nc.gpsimd.tensor_tensor(cb["prod"][64:128, :], cb["exq"][64:128, :],
                                                cb["zb2"][64:128, :], op=OP.mult)
                    elif step == 11:
                        nc.gpsimd.tensor_tensor(ctx_s[p_][cb["qc"]][:, :], cb["prod"][:],
                                                cb["xr"][:], op=OP.add)
                        cb["done"] = True

                N_CB_STEPS = 12
                pending_cb = None
                pending_av = None

                for qc in range(NQC):
                    for p in range(HPAIRS):
                        a0, a1 = 2 * p, 2 * p + 1
                        cols = bass.ts(qc, QCHUNK)
                        pse = acps.tile([128, 2 * QCHUNK], F32, tag="pe")
                        pse_a = pse[:, 0:QCHUNK]
                        pse_b = pse[:, QCHUNK:2 * QCHUNK]
                        psr = acps.tile([128, QCHUNK], F32, tag="pr")
                        pending = {}
                        pending_r = {}
                        blk = {"pse": pse, "psr": psr,
                               "a0": a0, "a1": a1, "pending": pending,
                               "pending_r": pending_r}

                        def av_e_for(bk, j):
                            # fp8 DoubleRow e-AV over k-tile pair (2j, 2j+1)
                            ep = bk["pending"].pop(j)
                            st, sp = j == 0, j == KTILES // 2 - 1
                            b0, b1 = bk["a0"], bk["a1"]
                            e3 = ep[:, :].rearrange("p (ko x) -> p ko x", ko=2)
                            v3 = vp_s[j][:, :].rearrange("p (ko x) -> p ko x", ko=2)
                            va = v3[:, :, b0 * DHP:b0 * DHP + DH + 1]
                            vb = v3[:, :, b1 * DHP:b1 * DHP + DH + 1]
                            nc.tensor.matmul(bk["pse"][0:DH + 1, 0:QCHUNK], va,
                                             e3[:, :, 0:QCHUNK],
                                             start=st, stop=sp, perf_mode=PM.DoubleRow)
                            nc.tensor.matmul(bk["pse"][0:DH + 1, QCHUNK:2 * QCHUNK], vb,
                                             e3[:, :, QCHUNK:2 * QCHUNK],
                                             start=st, stop=sp, perf_mode=PM.DoubleRow)

                        def av_r_for(bk, t):
                            # bf16 r-AV for k-tile t (col-packed head pair)
                            rt = bk["pending_r"].pop(t)
                            st, sp = t == 0, t == KTILES - 1
                            b0, b1 = bk["a0"], bk["a1"]
                            j, ph = t // 2, t % 2
                            v3 = vb_s[j][:, :].rearrange("p (ko x) -> p ko x", ko=2)
                            nc.tensor.matmul(bk["psr"][0:64, :],
                                             v3[:, ph:ph + 1, b0 * DH:(b0 + 1) * DH],
                                             rt[:, 0:QCHUNK], start=st, stop=sp)
                            nc.tensor.matmul(bk["psr"][64:128, :],
                                             v3[:, ph:ph + 1, b1 * DH:(b1 + 1) * DH],
                                             rt[:, QCHUNK:2 * QCHUNK], start=st, stop=sp)

                        for t in range(KTILES):
                            kcols = slice(S + t * 128, S + (t + 1) * 128)
                            qcols = slice(qc * QCHUNK, (qc + 1) * QCHUNK)
                            # both heads' score tiles side by side in one 2-bank
                            # PSUM tile; the two matmuls row-pack (tile_position
                            # (0,0) and (64,0) via base partitions)
                            ss = scps.tile([128, 2 * QCHUNK], F32, tag="s")
                            nc.tensor.matmul(ss[:, 0:QCHUNK], qk_s[p][0:64, kcols],
                                             qk_s[p][0:64, qcols])
                            nc.tensor.matmul(ss[:, QCHUNK:2 * QCHUNK], qk_s[p][64:128, kcols],
                                             qk_s[p][64:128, qcols])

                            # e pair tiles fp8 (ko-major halves); r tiles bf16
                            if t % 2 == 0:
                                etp = ewsb.tile([128, 4 * QCHUNK], F8, tag="e", name="e")
                                pending[t // 2] = etp
                            half = slice((t % 2) * 2 * QCHUNK, (t % 2 + 1) * 2 * QCHUNK)
                            rt = ewsb.tile([128, 2 * QCHUNK], F16, tag="r", name="r")
                            pending_r[t] = rt
                            nc.scalar.activation(etp[:, half], ss[:], AF.Exp)
                            nc.vector._custom_dve(RELU_SQ, out=rt[:], in0=ss[:])
                            # previous block's tail AVs, spread over early slots
                            if pending_av is not None:
                                bk_p, etail, rtail = pending_av
                                if t < len(etail):
                                    av_e_for(bk_p, etail[t])
                                if t < len(rtail):
                                    av_r_for(bk_p, rtail[t])
                            # previous block's combine, one micro-op per tile
                            cb_step(pending_cb, t - CB_OFF)
                            if t >= AVP_OFF and (t - AVP_OFF) % 2 == 0:
                                av_e_for(blk, (t - AVP_OFF) // 2)
                            if t >= RV_OFF:
                                av_r_for(blk, t - RV_OFF)
                            # next q-chunk's Q projection, one chain per block
                            if t == 5 and qc + 1 < NQC:
                                q_chain(p, qc + 1)
                            # pending outproj o-tiles mid-block
                            if p != 0 and t in OUTPROJ_SLOTS and pend_outproj:
                                qc_o, ot_o = pend_outproj.pop(0)
                                outproj_tile(qc_o, ot_o)
                        pending_av_next = (blk, sorted(pending), sorted(pending_r))

                        # defer this block's tail AVs and combine into the
                        # next block's loop
                        pending_av = pending_av_next
                        pending_cb = {"p": p, "pse": pse,
                                      "psr": psr, "cols": cols, "qc": qc}

                        # queue this q-chunk's output projection once all head
                        # pairs are done; emitted interleaved in later blocks
                        if p == HPAIRS - 1:
                            for ot in range(OTILES):
                                pend_outproj.append((qc, ot))

                # drain the last block's tail AVs and combine
                if pending_av is not None:
                    for j in pending_av[1]:
                        av_e_for(pending_av[0], j)
                    for tt in pending_av[2]:
                        av_r_for(pending_av[0], tt)
                for st_i in range(N_CB_STEPS):
                    cb_step(pending_cb, st_i)

        pend_outproj: list = []
        with tc.tile_pool(name="outps", bufs=1, space="PSUM") as outpool, \
             tc.tile_pool(name="obsb", bufs=2) as obsb:
            for _rep in range(repeat):
                phases(pend_outproj, outpool, obsb, _rep)
            # tail: remaining outproj tiles of the last rep (phase pools are
            # closed here, so banks are free for a wider tail pool)
            with tc.tile_pool(name="tailps", bufs=3, space="PSUM") as tailpool:
                while pend_outproj:
                    qc, ot = pend_outproj.pop(0)
                    cols = bass.ts(qc, QCHUNK)
                    pso = tailpool.tile([128, QCHUNK], F32, tag="pso",
                                        name=f"pso_tail_{qc}_{ot}")
                    orows = bass.ts(ot, 128)
                    for c in range(HPAIRS):
                        nc.tensor.matmul(pso[:], wo_t[c][:, orows],
                                         ctx_s[c][qc][:, :],
                                         start=(c == 0), stop=(c == HPAIRS - 1))
                    ob = obsb.tile([128, QCHUNK], F16, tag="ob",
                                   name=f"ob_tail_{qc}_{ot}")
                    if ot % 2 == 0:
                        nc.scalar.activation(ob[:], pso[:], AF.Copy)
                    else:
                        nc.vector.tensor_copy(ob[:], pso[:])
                    if OUT_DMA == "gp":
                        nc.gpsimd.dma_start(out[qc * OTILES + ot, :, :], ob[:])
                    elif OUT_DMA != "none":
                        nc.sync.dma_start(out[qc * OTILES + ot, :, :], ob[:])

    nc.compile()
    return nc


def _get_kernel(m0: float, m1: float, has_bias: bool):
    key = (round(m0, 9), round(m1, 9), has_bias)
    if key not in _KERNEL_CACHE:
        _KERNEL_CACHE[key] = build_kernel(m0, m1, has_bias)
    return _KERNEL_CACHE[key]


def make_in_maps(inputs: dict) -> tuple[list[dict], float, float, bool]:
    hidden = np.asarray(inputs["hidden_states"], dtype=np.float32)
    Wq = np.asarray(inputs["Wq"], dtype=np.float32)
    Wk = np.asarray(inputs["Wk"], dtype=np.float32)
    Wv = np.asarray(inputs["Wv"], dtype=np.float32)
    Wo = np.asarray(inputs["Wo"], dtype=np.float32)
    bq = np.asarray(inputs["bq"], dtype=np.float32)
    bk = np.asarray(inputs["bk"], dtype=np.float32)
    bv = np.asarray(inputs["bv"], dtype=np.float32)
    w_mix = np.asarray(inputs["w_mix"], dtype=np.float32)

    e = np.exp(w_mix - w_mix.max())
    mix = e / e.sum()
    m0, m1 = float(mix[0]), float(mix[1])
    has_bias = bool(bq.any() or bk.any() or bv.any())

    qk_scale = 1.0 / float(np.sqrt(DH))

    def bf(x):
        return np.ascontiguousarray(x).astype(NP_F16)

    in_maps = []
    for core in range(NCORES):
        b, g = core // 2, core % 2
        rows = slice(DLOC * g, DLOC * (g + 1))
        m = {
            "hT": bf(hidden[b].T),
            "wqT": bf(Wq[rows].T * qk_scale),
            "wkT": bf(Wk[rows].T),
            "wvT": bf(Wv[rows].T),
            "woT": bf(Wo[:, rows].T),
        }
        if has_bias:
            m["hb"] = bf(np.ones((1, S), dtype=np.float32))
            m["wqb"] = bf(bq[rows][None, :] * qk_scale)
            m["wkb"] = bf(bk[rows][None, :])
            m["wvb"] = bf(bv[rows][None, :])
        in_maps.append(m)
    return in_maps, m0, m1, has_bias


def assemble_output(results: list[dict], bo: np.ndarray) -> np.ndarray:
    out = np.empty((B, S, D), dtype=np.float32)
    for b in range(B):
        # [NQC*OTILES, 128, QCHUNK] tile-major -> [D, S]
        acc = (results[2 * b]["out"].astype(np.float32) +
               results[2 * b + 1]["out"].astype(np.float32))
        full = acc.reshape(NQC, OTILES, 128, QCHUNK).transpose(1, 2, 0, 3) \
                  .reshape(D, S)
        out[b] = full.T
    if bo.any():
        out += bo
    return out


def _spot_check(out: np.ndarray, inputs: dict, rng: np.random.Generator) -> bool:
    """Recompute one random query row per batch on the host (covers all 8
    cores' partial outputs) and compare; guards against transient HW faults."""
    hidden = np.asarray(inputs["hidden_states"], dtype=np.float32)
    Wq = np.asarray(inputs["Wq"], dtype=np.float32)
    Wk = np.asarray(inputs["Wk"], dtype=np.float32)
    Wv = np.asarray(inputs["Wv"], dtype=np.float32)
    Wo = np.asarray(inputs["Wo"], dtype=np.float32)
    bq = np.asarray(inputs["bq"], dtype=np.float32)
    bk = np.asarray(inputs["bk"], dtype=np.float32)
    bv = np.asarray(inputs["bv"], dtype=np.float32)
    bo = np.asarray(inputs["bo"], dtype=np.float32)
    w_mix = np.asarray(inputs["w_mix"], dtype=np.float32)
    e = np.exp(w_mix - w_mix.max())
    m0, m1 = e / e.sum()
    for b in range(B):
        s = int(rng.integers(0, S))
        q = (hidden[b, s] @ Wq.T + bq).reshape(H, DH) / np.sqrt(DH)
        k = (hidden[b] @ Wk.T + bk).reshape(S, H, DH)
        v = (hidden[b] @ Wv.T + bv).reshape(S, H, DH)
        scores = np.einsum("hd,khd->hk", q, k)
        sm = np.exp(scores - scores.max(axis=1, keepdims=True))
        sm /= sm.sum(axis=1, keepdims=True)
        attn = m0 * sm + m1 * np.maximum(scores, 0.0) ** 2
        ctx = np.einsum("hk,khd->hd", attn, v).reshape(D)
        want = ctx @ Wo.T + bo
        got = out[b, s]
        rel = np.abs(got - want).max() / max(np.abs(want).max(), 1e-6)
        if not np.isfinite(got).all() or rel > 0.05:
            return False
    return True


def kernel(**inputs) -> np.ndarray:
    in_maps, m0, m1, has_bias = make_in_maps(inputs)
    nc = _get_kernel(m0, m1, has_bias)
    bo = np.asarray(inputs["bo"], dtype=np.float32)
    rng = np.random.default_rng(12345)
    out = None
    for _attempt in range(3):
        res = run_bass_kernel_spmd(nc, in_maps, core_ids=list(range(NCORES)))
        out = assemble_output(res.results, bo)
        if np.isfinite(out).all() and _spot_check(out, inputs, rng):
            return out
    return out


# revision 38
# speedup vs baseline: 3.5318x; 1.4656x over previous
"""Trainium2 Bass kernel for mixed softmax + relu^2 attention (v2).

Reference computation (B=4, S=2048, D=768, H=12, DH=64):
    q = split_heads(hidden @ Wq.T + bq)        # [B,H,S,DH]
    k = split_heads(hidden @ Wk.T + bk)
    v = split_heads(hidden @ Wv.T + bv)
    scores = q @ k.T / sqrt(DH)                # [B,H,S,S]
    attn = m0 * softmax(scores) + m1 * relu(scores)^2,  (m0,m1) = softmax(w_mix)
    out = merge_heads(attn @ v) @ Wo.T + bo

Sharding over 8 NeuronCores: core = (batch b = core//2, head-group g = core%2 of
6 heads).  Each core computes its 6 heads' full SxS attention and a partial
output projection over its 384 context dims; the host sums the two partials
per batch.

Device-side layout ("transposed", k on partitions), per head pair p (2 heads
a0/a1 stacked on partitions 0-63 / 64-127):
  - qk[p] [128, 2S]: Q cols [0,S) (pre-scaled by 1/sqrt(DH) via host-side
    Wq scaling), K cols [S,2S).  Head-major rows.  Evicted from a single
    2-bank PSUM tile with one ACT copy per q-chunk.
  - scoresT tile ss [k=128, 2*512] = K_tile.T @ Q_chunk for both heads
    (row-packed concurrent matmuls via auto tile_position).
  - e = exp(ss) on ACT -> bf16; r = relu(ss)^2 on DVE (custom op) -> bf16.
  - V augmented per head: [alpha*V | beta] where alpha=max(m1,eps),
    beta=alpha/m0; e-AV accumulates [alpha*V|beta].T @ e so row 64 holds
    beta*Z (Z = softmax denominator); r-AV accumulates (alpha*V).T @ r
    col-packed for both heads into one psum tile.
  - combine: ACT evicts pse rows 0-64 -> SBUF; DVE reciprocal of the
    beta*Z rows (PSUM); GpSimd broadcasts 1/(beta*Z), multiplies and adds:
    ctx = ex * zb + xr  (equals m0*V.T e/Z + m1*V.T r by construction).
  - out_partial[o, s] = Wo_part.T @ ctx per 128-row o-tile, interleaved one
    o-tile per k-tile iteration of a later block; shipped fp32; host sums.
"""

from contextlib import ExitStack

import numpy as np
import ml_dtypes

import concourse.bass as bass
import concourse.mybir as mybir
import concourse.tile as tile
from concourse import bacc, dve_ops
from concourse.bass_utils import run_bass_kernel_spmd
from concourse.dve_spec import Spec, Src0, relu as _sp_relu, sq as _sp_sq


def _register_relu_sq():
    """Custom fused DVE op: out = relu(in0)^2 in a single pass."""
    for op in dve_ops.OPS:
        if op.name == "RELU_SQ_ANT":
            return op
    op = dve_ops.DveOp(
        "RELU_SQ_ANT",
        Spec(body=_sp_sq(_sp_relu(Src0)),
             reference=lambda in0: np.maximum(in0, 0.0) ** 2),
        subdim=False,
        uops_sha={"v3": "8abca05ebc329c1b", "v4": "4b83c053374efcdc"},
    )
    dve_ops.OPS.append(op)
    dve_ops.CUSTOM_DVE_SPECS[op.name] = op.spec
    dve_ops._SUB_OPCODE_FOR_NAME[op.name] = (
        dve_ops._CUSTOM_DVE_ROW_BASE + len(dve_ops.OPS) - 1
    )
    return op


RELU_SQ = _register_relu_sq()

B, S, D, H, DH = 4, 2048, 768, 12, 64
NCORES = 8
HL = H // 2          # local heads per core = 6
HPAIRS = HL // 2     # head pairs = 3
DLOC = HL * DH       # local context dims = 384
KTILES = S // 128    # 16
QCHUNK = 512
NQC = S // QCHUNK    # 4
DKT = D // 128       # 6 contraction tiles for projections
OTILES = D // 128    # 6 output-projection row tiles
DHP = 80             # padded per-head V block (DoubleRow needs ko stride %16==0)

F32 = mybir.dt.float32
BF16 = mybir.dt.bfloat16
F16 = mybir.dt.float16
F8 = mybir.dt.float8e4
PM = mybir.MatmulPerfMode
NP_BF16 = ml_dtypes.bfloat16
NP_F16 = np.float16
AF = mybir.ActivationFunctionType
OP = mybir.AluOpType

# AV pair j (k-tiles 2j,2j+1) issues at t-slot 2j + AVP_OFF, so the in-order
# PE stream never waits on the elementwise chain.  Pairs past the block end
# carry into the next block's first slots.
AVP_OFF = 9
# combine micro-ops start at this t-slot of the following block
CB_OFF = 4
# r-AV for k-tile t issues at t-slot t + RV_OFF
RV_OFF = 4
# engine for the outproj psum eviction: "act" | "dve" | "any"
OB_ENGINE = "act"
# engine for the psr psum eviction
XR_ENGINE = "dve"
# number of et/rt pair buffers
EW_BUFS = 6
import os
OUT_DMA = os.environ.get("OUT_DMA", "sp")
# k-tile slots at which outproj o-tiles are emitted (on p != 0 blocks only,
# so the newest ctx q-chunk has a full block of slack before first use)
OUTPROJ_SLOTS = (12, 14, 15)

_KERNEL_CACHE: dict = {}


def build_kernel(m0: float, m1: float, has_bias: bool, repeat: int = 1):
    nc = bacc.Bacc("TRN2", target_bir_lowering=False, debug=False)

    hT = nc.dram_tensor("hT", [D, S], F16, kind="ExternalInput").ap()
    wqT = nc.dram_tensor("wqT", [D, DLOC], F16, kind="ExternalInput").ap()
    wkT = nc.dram_tensor("wkT", [D, DLOC], F16, kind="ExternalInput").ap()
    wvT = nc.dram_tensor("wvT", [D, DLOC], F16, kind="ExternalInput").ap()
    woT = nc.dram_tensor("woT", [DLOC, D], F16, kind="ExternalInput").ap()
    if has_bias:
        hb = nc.dram_tensor("hb", [1, S], F16, kind="ExternalInput").ap()
        wqb = nc.dram_tensor("wqb", [1, DLOC], F16, kind="ExternalInput").ap()
        wkb = nc.dram_tensor("wkb", [1, DLOC], F16, kind="ExternalInput").ap()
        wvb = nc.dram_tensor("wvb", [1, DLOC], F16, kind="ExternalInput").ap()
    # tile-major output: each [128, QCHUNK] store is one contiguous block
    # (the [D, S] layout forced 128 separate 1KB row writes per DMA)
    out = nc.dram_tensor("out", [NQC * OTILES, 128, QCHUNK], F16,
                         kind="ExternalOutput").ap()

    # V is stored unscaled in fp8 (ones column exactly 1.0); the mix factors
    # are applied in the combine: zrec *= m0, and ctx = (xr * m1) + prod.

    with tile.TileContext(nc) as tc, ExitStack() as ctx:
        # ---------------- persistent SBUF ----------------
        pp = ctx.enter_context(tc.tile_pool(name="persist", bufs=1))

        h_t = [pp.tile([128, S], F16, tag=f"ht{k}", name=f"ht{k}") for k in range(DKT)]
        wq_t = [pp.tile([128, DLOC], F16, tag=f"wq{k}", name=f"wq{k}") for k in range(DKT)]
        wk_t = [pp.tile([128, DLOC], F16, tag=f"wk{k}", name=f"wk{k}") for k in range(DKT)]
        wv_t = [pp.tile([128, DLOC], F16, tag=f"wv{k}", name=f"wv{k}") for k in range(DKT)]
        wo_t = [pp.tile([128, D], F16, tag=f"wo{c}", name=f"wo{c}") for c in range(HPAIRS)]
        for k in range(DKT):
            nc.sync.dma_start(wk_t[k][:], wkT[k * 128:(k + 1) * 128, :])
            nc.sync.dma_start(h_t[k][:], hT[k * 128:(k + 1) * 128, :])
        for k in range(DKT):
            nc.sync.dma_start(wq_t[k][:], wqT[k * 128:(k + 1) * 128, :])
        for k in range(DKT):
            nc.sync.dma_start(wv_t[k][:], wvT[k * 128:(k + 1) * 128, :])
        for c in range(HPAIRS):
            nc.sync.dma_start(wo_t[c][:], woT[c * 128:(c + 1) * 128, :])
        if has_bias:
            hb_t = pp.tile([1, S], F16, tag="hbt")
            wqb_t = pp.tile([1, DLOC], F16, tag="wqbt")
            wkb_t = pp.tile([1, DLOC], F16, tag="wkbt")
            wvb_t = pp.tile([1, DLOC], F16, tag="wvbt")
            nc.sync.dma_start(hb_t[:], hb[:, :])
            nc.sync.dma_start(wqb_t[:], wqb[:, :])
            nc.sync.dma_start(wkb_t[:], wkb[:, :])
            nc.sync.dma_start(wvb_t[:], wvb[:, :])

        # Q and K side by side so one ACT copy evicts both per q-chunk
        qk_s = [pp.tile([128, 2 * S], F16, tag=f"qk{p}", name=f"qk{p}") for p in range(HPAIRS)]
        # V pair tiles for fp8 DoubleRow AV: vp_s[j] holds k-tiles 2j (ko=0)
        # and 2j+1 (ko=1); per head DHP cols = [V(64) | ones | pad]
        vp_s = [pp.tile([128, 2 * HL * DHP], F8, tag=f"vp{j}", name=f"vp{j}")
                for j in range(KTILES // 2)]
        # fp16 V copies for the r-branch AV (fp8 V costs ~1.5e-2 rel error)
        vb_s = [pp.tile([128, 2 * DLOC], F16, tag=f"vb{j}", name=f"vb{j}")
                for j in range(KTILES // 2)]
        # per-(p,qc) ctx tiles: avoids false tile-granular dependencies
        # between the combine write of one q-chunk and outproj reads of another
        ctx_s = [[pp.tile([128, QCHUNK], F16, tag=f"cx{p}_{q}", name=f"cx{p}_{q}")
                  for q in range(NQC)] for p in range(HPAIRS)]

        # ones columns are persistent: written once, never overwritten (the V
        # evictions write a strided AP that skips them)
        for j in range(KTILES // 2):
            vp4 = vp_s[j][:, :].rearrange("p (ko a d) -> p ko a d", ko=2, d=DHP)
            nc.gpsimd.memset(vp4[:, :, :, DH:DH + 1], 1.0)

        nkt = DKT + (1 if has_bias else 0)

        def ev_engine(name):
            if name == "act":
                return nc.scalar
            if name == "dve":
                return nc.vector
            return nc.any

        def phases(pend_outproj, outpool, obsb, rep):
            # emit one output-projection o-tile: pso accumulate over head
            # pairs, evict, dma
            def outproj_tile(qc, ot):
                cols = bass.ts(qc, QCHUNK)
                pso = outpool.tile([128, QCHUNK], F32, tag="pso",
                                   name=f"pso_r{rep}_{qc}_{ot}")
                orows = bass.ts(ot, 128)
                for c in range(HPAIRS):
                    nc.tensor.matmul(pso[:], wo_t[c][:, orows],
                                     ctx_s[c][qc][:, :],
                                     start=(c == 0), stop=(c == HPAIRS - 1))
                ob = obsb.tile([128, QCHUNK], F16, tag="ob",
                               name=f"ob_r{rep}_{qc}_{ot}")
                if ot % 2 == 0:
                    nc.scalar.activation(ob[:], pso[:], AF.Copy)
                else:
                    nc.vector.tensor_copy(ob[:], pso[:])
                if OUT_DMA == "gp":
                    nc.gpsimd.dma_start(out[qc * OTILES + ot, :, :], ob[:])
                elif OUT_DMA != "none":
                    nc.sync.dma_start(out[qc * OTILES + ot, :, :], ob[:])

            def drain_outproj(n):
                while len(pend_outproj) > n:
                    qc, ot = pend_outproj.pop(0)
                    outproj_tile(qc, ot)

            # ---------------- phase 2: attention (with fused prefix) ----
            with tc.tile_pool(name="scps", bufs=2, space="PSUM") as scps, \
                 tc.tile_pool(name="acps", bufs=1, space="PSUM") as acps, \
                 tc.tile_pool(name="ewsb", bufs=EW_BUFS) as ewsb, \
                 tc.tile_pool(name="cbsb", bufs=2) as cbsb:

                def q_chain(p_, qc_):
                    # one Q projection chain [128,512] via the aux psum bank
                    ps = outpool.tile([128, QCHUNK], F32, tag="pso", name="qch")
                    for k in range(nkt):
                        rhs = h_t[k][:, bass.ts(qc_, QCHUNK)] if k < DKT \
                            else hb_t[:, bass.ts(qc_, QCHUNK)]
                        wl = wq_t[k][:, p_ * 128:(p_ + 1) * 128] if k < DKT \
                            else wqb_t[:, p_ * 128:(p_ + 1) * 128]
                        nc.tensor.matmul(ps[:], wl, rhs, start=(k == 0),
                                         stop=(k == nkt - 1))
                    nc.scalar.activation(qk_s[p_][:, qc_ * QCHUNK:(qc_ + 1) * QCHUNK],
                                         ps[:], AF.Copy)

                # ---- prefix: all K, Q(q0), all V (scores need full K; AVs
                # need V; Q(qc>0) chains are interleaved into earlier blocks)
                drain_outproj(0)
                for p_ in range(HPAIRS):
                    for g in range(2):
                        ps = scps.tile([128, 2 * QCHUNK], F32, tag="s", name="kpre")
                        for half in range(2):
                            kc = 2 * g + half
                            dst = slice(half * QCHUNK, (half + 1) * QCHUNK)
                            for k in range(nkt):
                                rhs = h_t[k][:, bass.ts(kc, QCHUNK)] if k < DKT \
                                    else hb_t[:, bass.ts(kc, QCHUNK)]
                                wl = wk_t[k][:, p_ * 128:(p_ + 1) * 128] if k < DKT \
                                    else wkb_t[:, p_ * 128:(p_ + 1) * 128]
                                nc.tensor.matmul(ps[:, dst], wl, rhs, start=(k == 0),
                                                 stop=(k == nkt - 1))
                        nc.scalar.activation(
                            qk_s[p_][:, S + 2 * g * QCHUNK:S + 2 * (g + 1) * QCHUNK],
                            ps[:], AF.Copy)
                for p_ in range(HPAIRS):
                    q_chain(p_, 0)
                for g in range(KTILES // 2):
                    ps = scps.tile([128, 2 * QCHUNK], F32, tag="s", name="vpre")
                    vp4 = vp_s[g][:, :].rearrange("p (ko a d) -> p ko a d",
                                                  ko=2, d=DHP)
                    for half in range(2):
                        t_ = 2 * g + half
                        vdst = slice(half * QCHUNK, half * QCHUNK + DLOC)
                        for k in range(nkt):
                            lhsT = h_t[k][:, bass.ts(t_, 128)] if k < DKT \
                                else hb_t[:, bass.ts(t_, 128)]
                            rhs = wv_t[k][:] if k < DKT else wvb_t[:]
                            nc.tensor.matmul(ps[:, vdst], lhsT, rhs, start=(k == 0),
                                             stop=(k == nkt - 1))
                        psv_4d = ps[:, vdst].rearrange("p (ko a d) -> p ko a d",
                                                       ko=1, d=DH)
                        nc.scalar.activation(vp4[:, half:half + 1, :, 0:DH],
                                             psv_4d[:, :, :, :], AF.Copy)
                        vb3 = vb_s[g][:, :].rearrange("p (ko x) -> p ko x", ko=2)
                        nc.scalar.activation(vb3[:, half:half + 1, :], 
                                             ps[:, vdst].rearrange("p (ko x) -> p ko x", ko=1),
                                             AF.Copy)

                def cb_step(cb, step):
                    """One micro-op of the deferred combine for the previous
                    block; spread across the next block's k-tile loop so the
                    ACT/DVE FIFOs never see a burst."""
                    if cb is None:
                        return
                    p_, pse, psr_, cols_ = (
                        cb["p"], cb["pse"], cb["psr"], cb["cols"])
                    if step == 0:
                        # fold the relu^2 mix weight m1 into the psr eviction
                        cb["xr"] = cbsb.tile([128, QCHUNK], F32, tag="xr", name="xr")
                        if XR_ENGINE == "act":
                            nc.scalar.activation(cb["xr"][:], psr_[:], AF.Copy, scale=m1)
                        else:
                            nc.vector.tensor_scalar_mul(cb["xr"][:], psr_[:], m1)
                    elif step == 1:
                        # fold the softmax mix weight m0 into the pse eviction
                        cb["exq"] = cbsb.tile([128, QCHUNK], F32, tag="exq", name="exq")
                        nc.scalar.activation(cb["exq"][0:64, :], pse[0:64, 0:QCHUNK],
                                             AF.Copy, scale=m0)
                    elif step == 2:
                        # both heads' beta*Z rows in one op (pse halves adjacent)
                        cb["zrow"] = cbsb.tile([1, 2 * QCHUNK], F32, tag="zw", name="zw")
                        nc.scalar.activation(cb["zrow"][0:1, :], pse[64:65, :], AF.Copy)
                    elif step == 3:
                        nc.scalar.activation(cb["exq"][64:128, :],
                                             pse[0:64, QCHUNK:2 * QCHUNK],
                                             AF.Copy, scale=m0)
                    elif step == 5:
                        cb["zrec"] = cbsb.tile([1, 2 * QCHUNK], F32, tag="zr", name="zr")
                        nc.vector.reciprocal_approx_fast(cb["zrec"][:], cb["zrow"][:])
                    elif step == 7:
                        cb["zb1"] = cbsb.tile([128, QCHUNK], F32, tag="zb1", name="zb1")
                        nc.gpsimd.partition_broadcast(cb["zb1"][:, :],
                                                      cb["zrec"][0:1, 0:QCHUNK],
                                                      channels=128)
                    elif step == 8:
                        cb["zb2"] = cbsb.tile([128, QCHUNK], F32, tag="zb2", name="zb2")
                        nc.gpsimd.partition_broadcast(cb["zb2"][:, :],
                                                      cb["zrec"][0:1, QCHUNK:2 * QCHUNK],
                                                      channels=128)
                    elif step == 9:
                        cb["prod"] = cbsb.tile([128, QCHUNK], F32, tag="prod", name="prod")
                        nc.gpsimd.tensor_tensor(cb["prod"][0:64, :], cb["exq"][0:64, :],
                                                cb["zb1"][0:64, :], op=OP.mult)
                    elif step == 10:
                        nc.gpsimd.tensor_tensor(cb["prod"][64:128, :], cb["exq"][64:128, :],
                                                cb["zb2"][64:128, :], op=OP.mult)
                    elif step == 11:
                        nc.gpsimd.tensor_tensor(ctx_s[p_][cb["qc"]][:, :], cb["prod"][:],
                                                cb["xr"][:], op=OP.add)
                        cb["done"] = True

                N_CB_STEPS = 12
                pending_cb = None
                pending_av = None

                for qc in range(NQC):
                    for p in range(HPAIRS):
                        a0, a1 = 2 * p, 2 * p + 1
                        cols = bass.ts(qc, QCHUNK)
                        pse = acps.tile([128, 2 * QCHUNK], F32, tag="pe")
                        pse_a = pse[:, 0:QCHUNK]
                        pse_b = pse[:, QCHUNK:2 * QCHUNK]
                        psr = acps.tile([128, QCHUNK], F32, tag="pr")
                        pending = {}
                        pending_r = {}
                        blk = {"pse": pse, "psr": psr,
                               "a0": a0, "a1": a1, "pending": pending,
                               "pending_r": pending_r}

                        def av_e_for(bk, j):
                            # fp8 DoubleRow e-AV over k-tile pair (2j, 2j+1)
                            ep = bk["pending"].pop(j)
                            st, sp = j == 0, j == KTILES // 2 - 1
                            b0, b1 = bk["a0"], bk["a1"]
                            e3 = ep[:, :].rearrange("p (ko x) -> p ko x", ko=2)
                            v3 = vp_s[j][:, :].rearrange("p (ko x) -> p ko x", ko=2)
                            va = v3[:, :, b0 * DHP:b0 * DHP + DH + 1]
                            vb = v3[:, :, b1 * DHP:b1 * DHP + DH + 1]
                            nc.tensor.matmul(bk["pse"][0:DH + 1, 0:QCHUNK], va,
                                             e3[:, :, 0:QCHUNK],
                                             start=st, stop=sp, perf_mode=PM.DoubleRow)
                            nc.tensor.matmul(bk["pse"][0:DH + 1, QCHUNK:2 * QCHUNK], vb,
                                             e3[:, :, QCHUNK:2 * QCHUNK],
                                             start=st, stop=sp, perf_mode=PM.DoubleRow)

                        def av_r_for(bk, t):
                            # bf16 r-AV for k-tile t (col-packed head pair)
                            rt = bk["pending_r"].pop(t)
                            st, sp = t == 0, t == KTILES - 1
                            b0, b1 = bk["a0"], bk["a1"]
                            j, ph = t // 2, t % 2
                            v3 = vb_s[j][:, :].rearrange("p (ko x) -> p ko x", ko=2)
                            nc.tensor.matmul(bk["psr"][0:64, :],
                                             v3[:, ph:ph + 1, b0 * DH:(b0 + 1) * DH],
                                             rt[:, 0:QCHUNK], start=st, stop=sp)
                            nc.tensor.matmul(bk["psr"][64:128, :],
                                             v3[:, ph:ph + 1, b1 * DH:(b1 + 1) * DH],
                                             rt[:, QCHUNK:2 * QCHUNK], start=st, stop=sp)

                        for t in range(KTILES):
                            kcols = slice(S + t * 128, S + (t + 1) * 128)
                            qcols = slice(qc * QCHUNK, (qc + 1) * QCHUNK)
                            # both heads' score tiles side by side in one 2-bank
                            # PSUM tile; the two matmuls row-pack (tile_position
                            # (0,0) and (64,0) via base partitions)
                            ss = scps.tile([128, 2 * QCHUNK], F32, tag="s")
                            nc.tensor.matmul(ss[:, 0:QCHUNK], qk_s[p][0:64, kcols],
                                             qk_s[p][0:64, qcols])
                            nc.tensor.matmul(ss[:, QCHUNK:2 * QCHUNK], qk_s[p][64:128, kcols],
                                             qk_s[p][64:128, qcols])

                            # e pair tiles fp8 (ko-major halves); r tiles bf16
                            if t % 2 == 0:
                                etp = ewsb.tile([128, 4 * QCHUNK], F8, tag="e", name="e")
                                pending[t // 2] = etp
                            half = slice((t % 2) * 2 * QCHUNK, (t % 2 + 1) * 2 * QCHUNK)
                            rt = ewsb.tile([128, 2 * QCHUNK], F16, tag="r", name="r")
                            pending_r[t] = rt
                            nc.scalar.activation(etp[:, half], ss[:], AF.Exp)
                            nc.vector._custom_dve(RELU_SQ, out=rt[:], in0=ss[:])
                            # previous block's tail AVs, spread over early slots
                            if pending_av is not None:
                                bk_p, etail, rtail = pending_av
                                if t < len(etail):
                                    av_e_for(bk_p, etail[t])
                                if t < len(rtail):
                                    av_r_for(bk_p, rtail[t])
                            # previous block's combine, one micro-op per tile
                            cb_step(pending_cb, t - CB_OFF)
                            if t >= AVP_OFF and (t - AVP_OFF) % 2 == 0:
                                av_e_for(blk, (t - AVP_OFF) // 2)
                            if t >= RV_OFF:
                                av_r_for(blk, t - RV_OFF)
                            # next q-chunk's Q projection, one chain per block
                            if t == 5 and qc + 1 < NQC:
                                q_chain(p, qc + 1)
                            # pending outproj o-tiles mid-block
                            if p != 0 and t in OUTPROJ_SLOTS and pend_outproj:
                                qc_o, ot_o = pend_outproj.pop(0)
                                outproj_tile(qc_o, ot_o)
                        pending_av_next = (blk, sorted(pending), sorted(pending_r))

                        # defer this block's tail AVs and combine into the
                        # next block's loop
                        pending_av = pending_av_next
                        pending_cb = {"p": p, "pse": pse,
                                      "psr": psr, "cols": cols, "qc": qc}

                        # queue this q-chunk's output projection once all head
                        # pairs are done; emitted interleaved in later blocks
                        if p == HPAIRS - 1:
                            for ot in range(OTILES):
                                pend_outproj.append((qc, ot))

                # drain the last block's tail AVs and combine
                if pending_av is not None:
                    for j in pending_av[1]:
                        av_e_for(pending_av[0], j)
                    for tt in pending_av[2]:
                        av_r_for(pending_av[0], tt)
                for st_i in range(N_CB_STEPS):
                    cb_step(pending_cb, st_i)

        pend_outproj: list = []
        with tc.tile_pool(name="outps", bufs=1, space="PSUM") as outpool, \
             tc.tile_pool(name="obsb", bufs=2) as obsb:
            for _rep in range(repeat):
                phases(pend_outproj, outpool, obsb, _rep)
            # tail: remaining outproj tiles of the last rep (phase pools are
            # closed here, so banks are free for a wider tail pool)
            with tc.tile_pool(name="tailps", bufs=3, space="PSUM") as tailpool:
                while pend_outproj:
                    qc, ot = pend_outproj.pop(0)
                    cols = bass.ts(qc, QCHUNK)
                    pso = tailpool.tile([128, QCHUNK], F32, tag="pso",
                                        name=f"pso_tail_{qc}_{ot}")
                    orows = bass.ts(ot, 128)
                    for c in range(HPAIRS):
                        nc.tensor.matmul(pso[:], wo_t[c][:, orows],
                                         ctx_s[c][qc][:, :],
                                         start=(c == 0), stop=(c == HPAIRS - 1))
                    ob = obsb.tile([128, QCHUNK], F16, tag="ob",
                                   name=f"ob_tail_{qc}_{ot}")
                    if ot % 2 == 0:
                        nc.scalar.activation(ob[:], pso[:], AF.Copy)
                    else:
                        nc.vector.tensor_copy(ob[:], pso[:])
                    if OUT_DMA == "gp":
                        nc.gpsimd.dma_start(out[qc * OTILES + ot, :, :], ob[:])
                    elif OUT_DMA != "none":
                        nc.sync.dma_start(out[qc * OTILES + ot, :, :], ob[:])

    nc.compile()
    return nc


def _get_kernel(m0: float, m1: float, has_bias: bool):
    key = (round(m0, 9), round(m1, 9), has_bias)
    if key not in _KERNEL_CACHE:
        _KERNEL_CACHE[key] = build_kernel(m0, m1, has_bias)
    return _KERNEL_CACHE[key]


def make_in_maps(inputs: dict) -> tuple[list[dict], float, float, bool]:
    hidden = np.asarray(inputs["hidden_states"], dtype=np.float32)
    Wq = np.asarray(inputs["Wq"], dtype=np.float32)
    Wk = np.asarray(inputs["Wk"], dtype=np.float32)
    Wv = np.asarray(inputs["Wv"], dtype=np.float32)
    Wo = np.asarray(inputs["Wo"], dtype=np.float32)
    bq = np.asarray(inputs["bq"], dtype=np.float32)
    bk = np.asarray(inputs["bk"], dtype=np.float32)
    bv = np.asarray(inputs["bv"], dtype=np.float32)
    w_mix = np.asarray(inputs["w_mix"], dtype=np.float32)

    e = np.exp(w_mix - w_mix.max())
    mix = e / e.sum()
    m0, m1 = float(mix[0]), float(mix[1])
    has_bias = bool(bq.any() or bk.any() or bv.any())

    qk_scale = 1.0 / float(np.sqrt(DH))

    def bf(x):
        return np.ascontiguousarray(x).astype(NP_F16)

    in_maps = []
    for core in range(NCORES):
        b, g = core // 2, core % 2
        rows = slice(DLOC * g, DLOC * (g + 1))
        m = {
            "hT": bf(hidden[b].T),
            "wqT": bf(Wq[rows].T * qk_scale),
            "wkT": bf(Wk[rows].T),
            "wvT": bf(Wv[rows].T),
            "woT": bf(Wo[:, rows].T),
        }
        if has_bias:
            m["hb"] = bf(np.ones((1, S), dtype=np.float32))
            m["wqb"] = bf(bq[rows][None, :] * qk_scale)
            m["wkb"] = bf(bk[rows][None, :])
            m["wvb"] = bf(bv[rows][None, :])
        in_maps.append(m)
    return in_maps, m0, m1, has_bias


def assemble_output(results: list[dict], bo: np.ndarray) -> np.ndarray:
    out = np.empty((B, S, D), dtype=np.float32)
    for b in range(B):
        # [NQC*OTILES, 128, QCHUNK] tile-major -> [D, S]
        acc = (results[2 * b]["out"].astype(np.float32) +
               results[2 * b + 1]["out"].astype(np.float32))
        full = acc.reshape(NQC, OTILES, 128, QCHUNK).transpose(1, 2, 0, 3) \
                  .reshape(D, S)
        out[b] = full.T
    if bo.any():
        out += bo
    return out


def _spot_check(out: np.ndarray, inputs: dict, rng: np.random.Generator) -> bool:
    """Recompute one random query row per batch on the host (covers all 8
    cores' partial outputs) and compare; guards against transient HW faults."""
    hidden = np.asarray(inputs["hidden_states"], dtype=np.float32)
    Wq = np.asarray(inputs["Wq"], dtype=np.float32)
    Wk = np.asarray(inputs["Wk"], dtype=np.float32)
    Wv = np.asarray(inputs["Wv"], dtype=np.float32)
    Wo = np.asarray(inputs["Wo"], dtype=np.float32)
    bq = np.asarray(inputs["bq"], dtype=np.float32)
    bk = np.asarray(inputs["bk"], dtype=np.float32)
    bv = np.asarray(inputs["bv"], dtype=np.float32)
    bo = np.asarray(inputs["bo"], dtype=np.float32)
    w_mix = np.asarray(inputs["w_mix"], dtype=np.float32)
    e = np.exp(w_mix - w_mix.max())
    m0, m1 = e / e.sum()
    for b in range(B):
        s = int(rng.integers(0, S))
        q = (hidden[b, s] @ Wq.T + bq).reshape(H, DH) / np.sqrt(DH)
        k = (hidden[b] @ Wk.T + bk).reshape(S, H, DH)
        v = (hidden[b] @ Wv.T + bv).reshape(S, H, DH)
        scores = np.einsum("hd,khd->hk", q, k)
        sm = np.exp(scores - scores.max(axis=1, keepdims=True))
        sm /= sm.sum(axis=1, keepdims=True)
        attn = m0 * sm + m1 * np.maximum(scores, 0.0) ** 2
        ctx = np.einsum("hk,khd->hd", attn, v).reshape(D)
        want = ctx @ Wo.T + bo
        got = out[b, s]
        rel = np.abs(got - want).max() / max(np.abs(want).max(), 1e-6)
        if not np.isfinite(got).all() or rel > 0.05:
            return False
    return True


def kernel(**inputs) -> np.ndarray:
    in_maps, m0, m1, has_bias = make_in_maps(inputs)
    nc = _get_kernel(m0, m1, has_bias)
    bo = np.asarray(inputs["bo"], dtype=np.float32)
    rng = np.random.default_rng(12345)
    out = None
    for _attempt in range(3):
        res = run_bass_kernel_spmd(nc, in_maps, core_ids=list(range(NCORES)))
        out = assemble_output(res.results, bo)
        if np.isfinite(out).all() and _spot_check(out, inputs, rng):
            return out
    return out
